# revision 21
# baseline (speedup 1.0000x reference)
"""DGCNN (4-layer EdgeConv + head) Bass kernel for 8 Trainium2 NeuronCores.

Problem: nn_DGCNN_net (B=4, N=2048, K=32), eval-mode BN.

Sharding: 2 cores per batch element (B=4 x 2-way split of the N=2048 points).
Cores 2b,2b+1 process batch b; even core owns points 0..1023, odd core
1024..2047. After each of the first three EdgeConv layers the pair exchanges
its half of the new features (pair-wise AllGather); the global max-pool is
combined with a pair-wise AllReduce(max).

v2 design (channel-major, SBUF gather):
  - Everything stays channel-major [chan, points]: y = (s*Wnbr)@x is computed
    as yT [o, N] directly, the neighbor gather runs on GPSIMD via ap_gather
    (SBUF->SBUF free-axis gather, one instruction per 128-point block instead
    of 31 descriptor-generating indirect DMAs), and the EdgeConv output comes
    out channel-major, which is exactly the next layer's input layout - no
    transpose rebuild.
  - k-NN selection: pd[i,j] = 2 x_i.x_j - xx_j (the -xx_i row constant is
    dropped; it does not change the row-wise top-k).  The column index is
    packed into the low 11 mantissa bits of the fp32 pd value
    (packed = (pd & 0xFFFFF800) | j, one fused scalar_tensor_tensor reading
    the PSUM matmul result).  For same-sign floats bit-pattern order ==
    value order, so top-k of packed == top-k of pd quantized to ~2^-12
    relative - and the selected values carry their indices for free.
  - top-32 per row: 16x Max8 over 128-wide chunks -> 128-entry pool, then
    4 rounds of Max8 + 3 MatchReplace on the pool only.  (Exact unless a
    single 128-chunk holds >8 of the row's true top-32: P ~ 7e-4 per row.)
  - The [128,32] u32 index tile is re-laid into ap_gather's wrapped int16
    format [16, 2p+h] with one PE transpose + two selector matmuls + two
    strided fp32->int16 copies.
"""

import numpy as np

EPS = 1e-5
K = 32
N = 2048
B = 4
NCORES = 8
HALF = N // 2
NBLK = HALF // 128  # 8 point-blocks per core

# layer configs: (C_in, O_out)
LAYERS = [(3, 64), (64, 64), (64, 128), (128, 256)]

NEG_BIG = -3.0e38
IDX_MASK = 0xFFFFFF80  # clear low 7 mantissa bits (local idx)


def _build_program(n_cores: int, reduce_on_pool=(False, False, True, True)):
    import concourse.bass as bass
    import concourse.mybir as mybir
    import concourse.bacc as bacc
    import concourse.tile as tile
    from concourse.bass import ds, ts

    fp32 = mybir.dt.float32
    bf16 = mybir.dt.bfloat16
    u32 = mybir.dt.uint32
    i16 = mybir.dt.int16
    AF = mybir.ActivationFunctionType
    OP = mybir.AluOpType
    AX = mybir.AxisListType

    nc = bacc.Bacc(
        "TRN2",
        target_bir_lowering=False,
        debug=False,
        num_devices=n_cores,
    )

    # ---------------- external IO ----------------
    def din(name, shape, dt=fp32):
        return nc.dram_tensor(name, shape, dt, kind="ExternalInput")

    x0 = din("x0", [3, N])
    # per layer: wy [c, o] (NO bias), wz [c, o], bz [o, 1]
    wy_d = [din(f"wy{li}", [c, o]) for li, (c, o) in enumerate(LAYERS)]
    wz_d = [din(f"wz{li}", [c + 2 if li < 3 else c, o])
            for li, (c, o) in enumerate(LAYERS)]
    bz_d = din("bz3", [1, 256])  # L4 bias row (others folded into wz)
    w5 = din("w5", [512, 1024])
    b5r = din("b5r", [1, 1024])
    w6T = din("w6T", [5, 64])
    b6c = din("b6c", [64, 1])
    w7T = din("w7T", [7, 64])
    b7c = din("b7c", [64, 1])
    L1T = din("L1T", [1152, 512])
    b8r = din("b8r", [1, 512])
    L2T = din("L2T", [512, 256])
    b9r = din("b9r", [1, 256])
    L3T = din("L3T", [256, 28])
    bL3r = din("bL3r", [1, 28])
    lvec = din("lvec", [5, 1])
    nvec = din("nvec", [7, 1])
    ident = din("ident", [128, 128])
    iota_pat = din("iota_pat", [128, N], u32)   # column index, replicated
    mask_col = din("mask_col", [128, 1], u32)   # 0xFFFFF800
    MA_d = din("MA", [32, 128])                 # selector: row == r%16
    MB_d = din("MB", [32, 128])                 # selector: row == 16 + r%16

    out_t = nc.dram_tensor("out", [1, 28], fp32, kind="ExternalOutput")

    groups = [[2 * i, 2 * i + 1] for i in range(max(1, n_cores // 2))]

    with tile.TileContext(nc) as tc:
        pid = nc.partition_id()
        off = (pid & 1) * HALF          # this core's first point column
        other_off = HALF - off          # the pair core's first point column
        other_rank = 1 - (pid & 1)

        # ---------------- pools ----------------
        consts = tc.alloc_tile_pool(name="consts", bufs=1)
        xcmp = tc.alloc_tile_pool(name="xcmp", bufs=1)
        dramp = tc.alloc_tile_pool(name="dramp", bufs=1, space="DRAM")
        lw = tc.alloc_tile_pool(name="lw", bufs=1)      # layer-wide tiles
        pB = tc.alloc_tile_pool(name="pB", bufs=2)      # big per-block tiles
        pS = tc.alloc_tile_pool(name="pS", bufs=4)      # small per-block tiles
        psA = tc.alloc_tile_pool(name="psA", bufs=2, space="PSUM")  # pd chunks
        psM = tc.alloc_tile_pool(name="psM", bufs=2, space="PSUM")  # [128,512]
        psS = tc.alloc_tile_pool(name="psS", bufs=2, space="PSUM")  # [128,128]

        # ---------------- persistent consts ----------------
        ident_sb = consts.tile([128, 128], fp32, name="ident_sb")
        nc.sync.dma_start(ident_sb[:], ident[:, :])
        iota_sb = consts.tile([128, N], u32, name="iota_sb")
        nc.sync.dma_start(iota_sb[:], iota_pat[:, :])
        mask_sb = consts.tile([128, 1], u32, name="mask_sb")
        nc.sync.dma_start(mask_sb[:], mask_col[:, :])
        MA_sb = consts.tile([32, 128], fp32, name="MA_sb")
        nc.sync.dma_start(MA_sb[:], MA_d[:, :])
        MB_sb = consts.tile([32, 128], fp32, name="MB_sb")
        nc.sync.dma_start(MB_sb[:], MB_d[:, :])
        ones_col = consts.tile([128, 1], fp32, name="ones_col")
        nc.vector.memset(ones_col[:], 1.0)
        ones_row = consts.tile([1, 128], fp32, name="ones_row")
        nc.vector.memset(ones_row[:], 1.0)
        ones_half = consts.tile([1, HALF], fp32, name="ones_half")
        nc.vector.memset(ones_half[:], 1.0)
        neg1_row = consts.tile([1, N], fp32, name="neg1_row")
        nc.vector.memset(neg1_row[:], -1.0)

        # weights: L4 split into two 128-column halves
        wy_sb, wz_sb = [], []
        for li, (c, o) in enumerate(LAYERS):
            zr = c + 2 if li < 3 else c  # wz rows: [(Wc-Wn); 0; b] for L1-3
            if o <= 128:
                t1 = consts.tile([c, o], fp32, name=f"wy_sb{li}")
                nc.sync.dma_start(t1[:], wy_d[li][:, :])
                wy_sb.append([t1])
                t2 = consts.tile([zr, o], fp32, name=f"wz_sb{li}")
                nc.sync.dma_start(t2[:], wz_d[li][:, :])
                wz_sb.append([t2])
            else:
                ys, zs = [], []
                for h in range(o // 128):
                    t1 = consts.tile([c, 128], fp32, name=f"wy_sb{li}_{h}")
                    nc.sync.dma_start(t1[:], wy_d[li][:, ts(h, 128)])
                    ys.append(t1)
                    t2 = consts.tile([zr, 128], fp32, name=f"wz_sb{li}_{h}")
                    nc.sync.dma_start(t2[:], wz_d[li][:, ts(h, 128)])
                    zs.append(t2)
                wy_sb.append(ys)
                wz_sb.append(zs)
        bz4_sb = [consts.tile([1, 128], fp32, name=f"bz4_sb{h}")
                  for h in range(2)]
        for h in range(2):
            nc.sync.dma_start(bz4_sb[h][:], bz_d[:, ts(h, 128)])

        # channel-major layer inputs (x_cm[1..3] double as x1..x3 for head)
        x_cm = [
            xcmp.tile([c, N], fp32, name=f"x_cm{li}")
            for li, (c, o) in enumerate(LAYERS)
        ]
        nc.sync.dma_start(x_cm[0][:], x0[:, :])
        # layer-4 output, my half, channel-major
        x4_my = [xcmp.tile([128, HALF], fp32, name=f"x4_my{j}") for j in range(2)]

        # ---------------- DRAM scratch ----------------
        xchg_in = [
            dramp.tile([o, HALF], fp32, name=f"xchg_in{li}")
            for li, (c, o) in enumerate(LAYERS[:3])
        ]
        xchg_out = [
            dramp.tile([2 * o, HALF], fp32, name=f"xchg_out{li}")
            for li, (c, o) in enumerate(LAYERS[:3])
        ]
        hred_in = dramp.tile([128, 8], fp32, name="hred_in")
        hred_out = dramp.tile([128, 8], fp32, name="hred_out")

        # ================= EdgeConv layers =================
        def edge_layer(li, c, o):
            last = li == 3
            xc = x_cm[li]
            nh = len(wy_sb[li])  # number of 128-wide output halves

            # ---- layer-wide: xx row, -xx bt row, yT, negxx_full ----
            xx_row = lw.tile([1, N], fp32, tag="xx_row")
            for q in range(4):
                xsq = lw.tile([c, 512], fp32, tag="xsq", bufs=2)
                nc.scalar.activation(xsq[:], xc[:, ts(q, 512)], AF.Square)
                mm = psM.tile([128, 512], fp32, tag="mm512")
                nc.tensor.matmul(
                    mm[0:1, :], ones_col[0:c, :], xsq[:], start=True, stop=True
                )
                nc.scalar.copy(xx_row[:, ts(q, 512)], mm[0:1, :])

            # bt = [2x; -1; -xx] for L1-3 (c+2 <= 128 rows, exact -d^2 with
            # a_my = [x; xx; 1]); L4: bt = 2x only, the -xx_j arrives via a
            # PSUM prefill and -xx_i via a rank-1 accumulate.
            negxx_row = lw.tile([1, N], fp32, tag="negxx_row")
            nc.scalar.activation(negxx_row[:], xx_row[:], AF.Copy, scale=-1.0)
            if c + 2 <= 128:
                bt = lw.tile([c + 2, N], fp32, tag="bt")
                bt_main_rows = c + 2
            else:
                bt = lw.tile([c, N], fp32, tag="bt")
                bt_main_rows = c
            for q in range(4):
                nc.scalar.activation(
                    bt[0:c, ts(q, 512)], xc[:, ts(q, 512)], AF.Copy, scale=2.0
                )
            if bt_main_rows == c + 2:
                nc.sync.dma_start(bt[c:c + 1, :], neg1_row[:])
                nc.sync.dma_start(bt[c + 1:c + 2, :], negxx_row[:])
                negxx_full = None
            else:
                # L4: prefill tile -xx broadcast to 128 partitions via PE
                negxx_full = lw.tile([128, N], fp32, tag="negxx_full")
                for q in range(4):
                    mm = psM.tile([128, 512], fp32, tag="mm512")
                    nc.tensor.matmul(
                        mm[:], ones_row[:], negxx_row[:, ts(q, 512)],
                        start=True, stop=True,
                    )
                    nc.scalar.copy(negxx_full[:, ts(q, 512)], mm[:])

            # yT [o, N] channel-major (nh tiles of <=128 partitions)
            yT = []
            for h in range(nh):
                ow = wy_sb[li][h].shape[1]
                t = lw.tile([ow, N], fp32, tag=f"yT{h}")
                for q in range(4):
                    mm = psM.tile([128, 512], fp32, tag="mm512")
                    nc.tensor.matmul(
                        mm[0:ow, :], wy_sb[li][h][:], xc[:, ts(q, 512)],
                        start=True, stop=True,
                    )
                    nc.scalar.copy(t[:, ts(q, 512)], mm[0:ow, :])
                yT.append(t)

            # pd lhsT: [x; xx; 1] (c+2 rows) for L1-3; L4 uses [x] plus a
            # separate xx_my row (rank-1 -xx_i accumulate).  Copied to
            # static-offset tiles: matmul operands reject register offsets.
            a_my = lw.tile([bt_main_rows, HALF], fp32, tag="a_my")
            nc.sync.dma_start(a_my[0:c, :], xc[:, ds(off, HALF)])
            if bt_main_rows == c + 2:
                nc.sync.dma_start(a_my[c:c + 1, :], xx_row[:, ds(off, HALF)])
                nc.sync.dma_start(a_my[c + 1:c + 2, :], ones_half[:])
                xx_my = None
            else:
                xx_my = lw.tile([1, HALF], fp32, tag="xx_my")
                nc.sync.dma_start(xx_my[:], xx_row[:, ds(off, HALF)])

            # EdgeConv output (my half, channel-major) at static offsets
            if not last:
                xo_my = [
                    lw.tile([t.shape[0], HALF], fp32, tag=f"xo_my{h}",
                            name=f"xo_my{li}_{h}")
                    for h, t in enumerate(yT)
                ]
            else:
                xo_my = x4_my

            # ---- per point-block of my half ----
            for i in range(NBLK):
                xsl = a_my[0:c, ts(i, 128)]  # [c, 128] static-offset slice

                # pd chunks -> packed [128, N] (fused mask|iota from PSUM)
                packed = pB.tile([128, N], fp32, tag="packed")
                for q in range(4):
                    pd_ps = psA.tile([128, 512], fp32, tag="pd_ps")
                    if negxx_full is not None:
                        nc.scalar.copy(pd_ps[:], negxx_full[:, ts(q, 512)])
                        nc.tensor.matmul(
                            pd_ps[:], xsl, bt[:, ts(q, 512)],
                            start=False, stop=True,
                        )
                    else:
                        nc.tensor.matmul(
                            pd_ps[:], a_my[:, ts(i, 128)],
                            bt[:, ts(q, 512)], start=True, stop=True,
                        )
                    nc.vector.scalar_tensor_tensor(
                        packed[:, ts(q, 512)].bitcast(u32),
                        pd_ps[:].bitcast(u32), mask_sb[:],
                        iota_sb[:, ts(q, 512)],
                        op0=OP.bitwise_and, op1=OP.bitwise_or,
                    )

                # chunk-pool top-32
                pool = pS.tile([128, 128], fp32, tag="pool")
                for ch in range(16):
                    nc.vector.max(
                        pool[:, ts(ch, 8)], packed[:, ts(ch, 128)]
                    )
                If_t = pS.tile([128, 32], fp32, tag="If_t")
                for r in range(4):
                    v8 = pS.tile([128, 8], fp32, tag="v8", bufs=8)
                    nc.vector.max(v8[:], pool[:])
                    pos8 = pS.tile([128, 8], u32, tag="pos8", bufs=8)
                    nc.vector.max_index(pos8[:], v8[:], pool[:])
                    # global idx = (pool_pos >> 3) * 128 | (packed & 0x7F)
                    pa = pS.tile([128, 8], u32, tag="pa", bufs=8)
                    nc.vector.tensor_scalar(
                        pa[:], v8[:].bitcast(u32), 127, None,
                        op0=OP.bitwise_and,
                    )
                    pb = pS.tile([128, 8], u32, tag="pb", bufs=8)
                    nc.vector.tensor_scalar(
                        pb[:], pos8[:], 3, 7,
                        op0=OP.logical_shift_right, op1=OP.logical_shift_left,
                    )
                    pc = pS.tile([128, 8], u32, tag="pc", bufs=8)
                    nc.vector.tensor_tensor(pc[:], pb[:], pa[:], op=OP.bitwise_or)
                    nc.vector.tensor_copy(If_t[:, ts(r, 8)], pc[:])
                    if r < 3:
                        nc.vector.match_replace(pool[:], v8[:], pool[:], NEG_BIG)

                # index re-layout for ap_gather
                it_ps = psS.tile([128, 128], fp32, tag="mm128")
                nc.tensor.transpose(it_ps[0:32, :], If_t[:, :], ident_sb[:])
                it_sb = pS.tile([32, 128], fp32, tag="it_sb")
                nc.scalar.copy(it_sb[:], it_ps[0:32, :])
                A_ps = psS.tile([128, 128], fp32, tag="mm128")
                nc.tensor.matmul(A_ps[:], MA_sb[:], it_sb[:], start=True, stop=True)
                B_ps = psS.tile([128, 128], fp32, tag="mm128")
                nc.tensor.matmul(B_ps[:], MB_sb[:], it_sb[:], start=True, stop=True)
                idxs_t = pS.tile([128, 128, 2], i16, tag="idxs_t")
                nc.vector.tensor_copy(idxs_t[:, :, 0:1], A_ps[:, :])
                nc.vector.tensor_copy(idxs_t[:, :, 1:2], B_ps[:, :])

                # gather + reduce + epilogue per output half
                for h in range(nh):
                    ow = yT[h].shape[0]
                    g = pB.tile([128, 128, K], fp32, tag="g")
                    nc.gpsimd.ap_gather(
                        g[0:ow, :, :], yT[h][:, :], idxs_t[0:ow, :, :],
                        channels=ow, num_elems=N, d=1, num_idxs=128 * K,
                    )
                    # cast on the idle Act engine so the DVE reduce runs in
                    # bf16 2x mode (y only feeds the feature max + add)
                    gb = pB.tile([128, 128, K], bf16, tag="gb", bufs=1)
                    nc.scalar.copy(gb[0:ow, :, :], g[0:ow, :, :])
                    gmax = pS.tile([128, 128], bf16, tag="gmax", bufs=8)
                    nc.vector.tensor_reduce(
                        gmax[0:ow, :], gb[0:ow, :, :], axis=AX.X, op=OP.max
                    )
                    # zT (+bias) and epilogue
                    z_ps = psS.tile([128, 128], fp32, tag="mm128")
                    if li < 3:
                        nc.tensor.matmul(
                            z_ps[0:ow, :], wz_sb[li][h][:],
                            a_my[:, ts(i, 128)], start=True, stop=True,
                        )
                    else:
                        nc.tensor.matmul(
                            z_ps[0:ow, :], wz_sb[li][h][:], xsl,
                            start=True, stop=False,
                        )
                        nc.tensor.matmul(
                            z_ps[0:ow, :], bz4_sb[h][:], ones_row[:],
                            start=False, stop=True,
                        )
                    u_t = pS.tile([128, 128], fp32, tag="u_t", bufs=8)
                    nc.vector.tensor_tensor(
                        u_t[0:ow, :], gmax[0:ow, :], z_ps[0:ow, :], op=OP.add
                    )
                    nc.vector.scalar_tensor_tensor(
                        xo_my[h][ds(0, ow), ts(i, 128)], u_t[0:ow, :], 0.2,
                        u_t[0:ow, :], op0=OP.mult, op1=OP.max,
                    )

            if not last:
                # my half into x_cm[li+1] and the exchange buffer
                for h, t in enumerate(xo_my):
                    ow = t.shape[0]
                    nc.sync.dma_start(
                        x_cm[li + 1][ds(h * 128, ow), ds(off, HALF)], t[:]
                    )
                    nc.sync.dma_start(xchg_in[li][ds(h * 128, ow), :], t[:])
                # ---- pair AllGather; fill the other half of x_cm[li+1] ----
                if n_cores == 1:
                    nc.sync.dma_start(
                        xchg_out[li][0:o, :], xchg_in[li][:, :]
                    )
                    nc.sync.dma_start(
                        xchg_out[li][o:2 * o, :], xchg_in[li][:, :]
                    )
                else:
                    nc.gpsimd.collective_compute(
                        "AllGather",
                        mybir.AluOpType.bypass,
                        replica_groups=groups,
                        ins=[xchg_in[li][:, :]],
                        outs=[xchg_out[li][:, :]],
                    )
                nc.sync.dma_start(
                    x_cm[li + 1][:, ds(other_off, HALF)],
                    xchg_out[li][ds(other_rank * o, o), :],
                )

        for li, (c, o) in enumerate(LAYERS):
            edge_layer(li, c, o)

        # ================= head =================
        psS.release()
        psM.release()
        psA.release()
        pS.release()
        pB.release()
        lw.release()
        w1 = tc.alloc_tile_pool(name="hw1", bufs=1)
        psA2 = tc.alloc_tile_pool(name="hpsA", bufs=1, space="PSUM")
        psC2 = tc.alloc_tile_pool(name="hpsC", bufs=1, space="PSUM")

        # my-half slices of x1..x3 at static offsets (matmul lhsT constraint)
        x1_my = w1.tile([64, HALF], fp32, tag="x1_my")
        nc.sync.dma_start(x1_my[:], x_cm[1][:, ds(off, HALF)])
        x2_my = w1.tile([64, HALF], fp32, tag="x2_my")
        nc.sync.dma_start(x2_my[:], x_cm[2][:, ds(off, HALF)])
        x3_my = w1.tile([128, HALF], fp32, tag="x3_my")
        nc.sync.dma_start(x3_my[:], x_cm[3][:, ds(off, HALF)])

        w5_sb = []
        for k2, (r0, r1) in enumerate([(0, 64), (64, 128), (128, 256),
                                       (256, 384), (384, 512)]):
            t = w1.tile([r1 - r0, 1024], fp32, tag=f"w5_{k2}")
            nc.sync.dma_start(t[:], w5[r0:r1, :])
            w5_sb.append(t)

        hmax = w1.tile([128, 1024], fp32, tag="hmax")
        for i in range(8):
            h_ps = psA2.tile([128, 1024], fp32, tag="h_ps")
            lhs = [x1_my[:, ts(i, 128)],
                   x2_my[:, ts(i, 128)],
                   x3_my[:, ts(i, 128)],
                   x4_my[0][:, ts(i, 128)],
                   x4_my[1][:, ts(i, 128)]]
            for q in range(2):
                for ci, l_ap in enumerate(lhs):
                    nc.tensor.matmul(
                        h_ps[:, ts(q, 512)], l_ap,
                        w5_sb[ci][:, ts(q, 512)],
                        start=(ci == 0), stop=(ci == len(lhs) - 1),
                    )
            if i == 0:
                nc.scalar.copy(hmax[:], h_ps[:])
            else:
                nc.vector.tensor_tensor(hmax[:], h_ps[:], hmax[:], op=OP.max)

        # partition-reduce via transposes -> [128, 8] (chan 128*j+p at [p, j])
        hcat = w1.tile([128, 8], fp32, tag="hcat")
        for j in range(8):
            tp = psC2.tile([128, 128], fp32, tag="tp")
            nc.tensor.transpose(tp[:], hmax[:, ts(j, 128)], ident_sb[:])
            nc.vector.tensor_reduce(
                hcat[:, j:j + 1], tp[:], axis=AX.X, op=OP.max
            )
        nc.sync.dma_start(hred_in[:, :], hcat[:])
        if n_cores == 1:
            nc.sync.dma_start(hred_out[:, :], hred_in[:, :])
        else:
            nc.gpsimd.collective_compute(
                "AllReduce", OP.max, replica_groups=groups,
                ins=[hred_in[:, :]], outs=[hred_out[:, :]],
            )
        hfull = w1.tile([128, 8], fp32, tag="hfull")
        nc.sync.dma_start(hfull[:], hred_out[:, :])
        b5_sb = w1.tile([128, 8], fp32, tag="b5_sb")
        nc.sync.dma_start(
            b5_sb[:], b5r.ap().rearrange("o (j p) -> (o p) j", p=128)
        )
        nc.vector.tensor_tensor(hfull[:], hfull[:], b5_sb[:], op=OP.add)
        nc.vector.scalar_tensor_tensor(
            hfull[:], hfull[:], 0.2, hfull[:], op0=OP.mult, op1=OP.max
        )

        # lf / nf columns
        lvec_sb = w1.tile([5, 1], fp32, tag="lvec_sb")
        nc.sync.dma_start(lvec_sb[:], lvec[:, :])
        nvec_sb = w1.tile([7, 1], fp32, tag="nvec_sb")
        nc.sync.dma_start(nvec_sb[:], nvec[:, :])
        w6_sb = w1.tile([5, 64], fp32, tag="w6_sb")
        nc.sync.dma_start(w6_sb[:], w6T[:, :])
        w7_sb = w1.tile([7, 64], fp32, tag="w7_sb")
        nc.sync.dma_start(w7_sb[:], w7T[:, :])
        b6_sb = w1.tile([64, 1], fp32, tag="b6_sb")
        nc.sync.dma_start(b6_sb[:], b6c[:, :])
        b7_sb = w1.tile([64, 1], fp32, tag="b7_sb")
        nc.sync.dma_start(b7_sb[:], b7c[:, :])

        def matvec_col(w_sb, v_sb, b_sb, n_out, tag):
            ps = psC2.tile([n_out, 1], fp32, tag="tpv")
            nc.tensor.matmul(ps[:], w_sb[:], v_sb[:], start=True, stop=True)
            r = w1.tile([n_out, 1], fp32, tag=tag)
            nc.vector.tensor_tensor(r[:], ps[:], b_sb[:], op=OP.add)
            nc.vector.scalar_tensor_tensor(
                r[:], r[:], 0.2, r[:], op0=OP.mult, op1=OP.max
            )
            return r

        lf_sb = matvec_col(w6_sb, lvec_sb, b6_sb, 64, "lf_sb")
        nf_sb = matvec_col(w7_sb, nvec_sb, b7_sb, 64, "nf_sb")

        # u tile [128, 9]: cols 0..7 = h, col 8 = [lf ; nf]
        u_t = w1.tile([128, 9], fp32, tag="u_t")
        nc.vector.tensor_copy(u_t[:, 0:8], hfull[:])
        nc.sync.dma_start(u_t[0:64, 8:9], lf_sb[:])
        nc.sync.dma_start(u_t[64:128, 8:9], nf_sb[:])

        def fc_row(v_cols, n_ch, wT_d, n_out, b_d, relu, tag):
            """out [1, n_out] = v.T @ wT ; v given as [128, n_ch] columns."""
            w_sb = w1.tile([128, n_ch, n_out], fp32, tag=f"{tag}_w")
            nc.sync.dma_start(
                w_sb[:], wT_d.ap().rearrange("(ch p) f -> p ch f", p=128)
            )
            ps = psC2.tile([1, n_out], fp32, tag="fcps")
            for ch in range(n_ch):
                nc.tensor.matmul(
                    ps[:], v_cols[:, ch:ch + 1], w_sb[:, ch, :],
                    start=(ch == 0), stop=(ch == n_ch - 1),
                )
            b_sb = w1.tile([1, n_out], fp32, tag=f"{tag}_b")
            nc.sync.dma_start(b_sb[:], b_d[:, :])
            r = w1.tile([1, n_out], fp32, tag=f"{tag}_r")
            nc.vector.tensor_tensor(r[:], ps[:], b_sb[:], op=OP.add)
            if relu:
                nc.vector.tensor_scalar_max(r[:], r[:], 0.0)
            return r

        def row_to_cols(v_row, n_ch, tag):
            """[1, 128*n_ch] -> [128, n_ch] via PE transposes."""
            cols = w1.tile([128, n_ch], fp32, tag=tag)
            for j in range(n_ch):
                tpv = psC2.tile([128, 1], fp32, tag="tpv2")
                nc.tensor.transpose(
                    tpv[:], v_row[:, ts(j, 128)], ident_sb[0:1, 0:1]
                )
                nc.vector.tensor_copy(cols[:, j:j + 1], tpv[:])
            return cols

        v1 = fc_row(u_t, 9, L1T, 512, b8r, True, "fc1")
        v1c = row_to_cols(v1, 4, "v1c")
        v2 = fc_row(v1c, 4, L2T, 256, b9r, True, "fc2")
        v2c = row_to_cols(v2, 2, "v2c")
        v3 = fc_row(v2c, 2, L3T, 28, bL3r, False, "fc3")
        nc.sync.dma_start(out_t[:, :], v3[:])

        for p in (psC2, psA2, w1, dramp, xcmp, consts):
            p.release()

    nc.compile()
    return nc


_PROGRAM_CACHE = {}


def get_program(n_cores=NCORES):
    key = n_cores
    if key not in _PROGRAM_CACHE:
        _PROGRAM_CACHE[key] = _build_program(n_cores)
    return _PROGRAM_CACHE[key]


def make_in_maps(inputs, n_cores=NCORES):
    """Host-side preprocessing: fold BN into weights, build per-core inputs."""
    f32 = np.float32

    def arr(v):
        return np.ascontiguousarray(np.asarray(v), dtype=f32)

    x = arr(inputs["x"])  # [B, 3, N]
    lmat = arr(inputs["l"])  # [B, 5]
    nmat = arr(inputs["n"])  # [B, 7]

    def fold(g):
        return arr(g) / np.sqrt(f32(1.0) + f32(EPS), dtype=f32)

    common = {}
    for li, (wn, gn, bn) in enumerate(
        [("W1", "g1", "b1"), ("W2", "g2", "b2"), ("W3", "g3", "b3"),
         ("W4", "g4", "b4")]
    ):
        W = arr(inputs[wn])  # [O, 2C]
        s = fold(inputs[gn])
        b = arr(inputs[bn])
        C = W.shape[1] // 2
        Wn = W[:, :C] * s[:, None]
        Wc = W[:, C:] * s[:, None]
        common[f"wy{li}"] = arr(Wn.T)
        wzT = arr((Wc - Wn).T)
        if li < 3:
            common[f"wz{li}"] = arr(np.concatenate(
                [wzT, np.zeros((1, len(b)), f32), b[None, :]], axis=0))
        else:
            common[f"wz{li}"] = wzT
            common["bz3"] = arr(b)[None, :]

    s5 = fold(inputs["g5"])
    common["w5"] = arr((arr(inputs["W5"]) * s5[:, None]).T)
    common["b5r"] = arr(inputs["b5"])[None, :]
    s6 = fold(inputs["g6"])
    common["w6T"] = arr((arr(inputs["W6"]) * s6[:, None]).T)
    common["b6c"] = arr(inputs["b6"])[:, None]
    s7 = fold(inputs["g7"])
    common["w7T"] = arr((arr(inputs["W7"]) * s7[:, None]).T)
    common["b7c"] = arr(inputs["b7"])[:, None]
    s8 = fold(inputs["g8"])
    common["L1T"] = arr((arr(inputs["L1"]) * s8[:, None]).T)
    common["b8r"] = arr(inputs["b8"])[None, :]
    s9 = fold(inputs["g9"])
    common["L2T"] = arr((arr(inputs["L2"]) * s9[:, None]).T)
    common["b9r"] = arr(s9 * arr(inputs["L2b"]) + arr(inputs["b9"]))[None, :]
    common["L3T"] = arr(arr(inputs["L3"]).T)
    common["bL3r"] = arr(inputs["L3b"])[None, :]
    common["ident"] = np.eye(128, dtype=f32)
    common["iota_pat"] = np.ascontiguousarray(
        np.broadcast_to(np.arange(N, dtype=np.uint32) % 128, (128, N))
    )
    common["mask_col"] = np.full((128, 1), IDX_MASK, dtype=np.uint32)
    MA = np.zeros((32, 128), f32)
    MB = np.zeros((32, 128), f32)
    for r in range(128):
        MA[r % 16, r] = 1.0
        MB[16 + r % 16, r] = 1.0
    common["MA"] = MA
    common["MB"] = MB

    in_maps = []
    for core in range(n_cores):
        b_i = (core // 2) % B
        m = dict(common)
        m["x0"] = arr(x[b_i])
        m["lvec"] = arr(lmat[b_i])[:, None]
        m["nvec"] = arr(nmat[b_i])[:, None]
        in_maps.append(m)
    return in_maps


LAST_RESULTS = None


def kernel(**inputs):
    global LAST_RESULTS
    from concourse.bass_utils import run_bass_kernel_spmd

    nc = get_program(NCORES)
    in_maps = make_in_maps(inputs, NCORES)
    res = run_bass_kernel_spmd(nc, in_maps, core_ids=list(range(NCORES)))
    LAST_RESULTS = res
    rows = [res.results[2 * b]["out"].reshape(28) for b in range(B)]
    return np.stack(rows, axis=0).astype(np.float32)


# revision 22
# speedup vs baseline: 1.0353x; 1.0353x over previous
"""DGCNN (4-layer EdgeConv + head) Bass kernel for 8 Trainium2 NeuronCores.

Problem: nn_DGCNN_net (B=4, N=2048, K=32), eval-mode BN.

Sharding: 2 cores per batch element (B=4 x 2-way split of the N=2048 points).
Cores 2b,2b+1 process batch b; even core owns points 0..1023, odd core
1024..2047. After each of the first three EdgeConv layers the pair exchanges
its half of the new features (pair-wise AllGather); the global max-pool is
combined with a pair-wise AllReduce(max).

v2 design (channel-major, SBUF gather):
  - Everything stays channel-major [chan, points]: y = (s*Wnbr)@x is computed
    as yT [o, N] directly, the neighbor gather runs on GPSIMD via ap_gather
    (SBUF->SBUF free-axis gather, one instruction per 128-point block instead
    of 31 descriptor-generating indirect DMAs), and the EdgeConv output comes
    out channel-major, which is exactly the next layer's input layout - no
    transpose rebuild.
  - k-NN selection: pd[i,j] = 2 x_i.x_j - xx_j (the -xx_i row constant is
    dropped; it does not change the row-wise top-k).  The column index is
    packed into the low 11 mantissa bits of the fp32 pd value
    (packed = (pd & 0xFFFFF800) | j, one fused scalar_tensor_tensor reading
    the PSUM matmul result).  For same-sign floats bit-pattern order ==
    value order, so top-k of packed == top-k of pd quantized to ~2^-12
    relative - and the selected values carry their indices for free.
  - top-32 per row: 16x Max8 over 128-wide chunks -> 128-entry pool, then
    4 rounds of Max8 + 3 MatchReplace on the pool only.  (Exact unless a
    single 128-chunk holds >8 of the row's true top-32: P ~ 7e-4 per row.)
  - The [128,32] u32 index tile is re-laid into ap_gather's wrapped int16
    format [16, 2p+h] with one PE transpose + two selector matmuls + two
    strided fp32->int16 copies.
"""

import numpy as np

EPS = 1e-5
K = 32
N = 2048
B = 4
NCORES = 8
HALF = N // 2
NBLK = HALF // 128  # 8 point-blocks per core

# layer configs: (C_in, O_out)
LAYERS = [(3, 64), (64, 64), (64, 128), (128, 256)]

NEG_BIG = -3.0e38
IDX_MASK = 0xFFFFFF80  # clear low 7 mantissa bits (local idx)


def _build_program(n_cores: int, reduce_on_pool=(False, False, True, True)):
    import concourse.bass as bass
    import concourse.mybir as mybir
    import concourse.bacc as bacc
    import concourse.tile as tile
    from concourse.bass import ds, ts

    fp32 = mybir.dt.float32
    bf16 = mybir.dt.bfloat16
    u32 = mybir.dt.uint32
    i16 = mybir.dt.int16
    AF = mybir.ActivationFunctionType
    OP = mybir.AluOpType
    AX = mybir.AxisListType

    nc = bacc.Bacc(
        "TRN2",
        target_bir_lowering=False,
        debug=False,
        num_devices=n_cores,
    )

    # ---------------- external IO ----------------
    def din(name, shape, dt=fp32):
        return nc.dram_tensor(name, shape, dt, kind="ExternalInput")

    x0 = din("x0", [3, N])
    # per layer: wy [c, o] (NO bias), wz [c, o], bz [o, 1]
    wy_d = [din(f"wy{li}", [c, o]) for li, (c, o) in enumerate(LAYERS)]
    wz_d = [din(f"wz{li}", [c + 2 if li < 3 else c, o])
            for li, (c, o) in enumerate(LAYERS)]
    bz_d = din("bz3", [1, 256])  # L4 bias row (others folded into wz)
    w5 = din("w5", [512, 1024])
    b5r = din("b5r", [1, 1024])
    w6T = din("w6T", [5, 64])
    b6c = din("b6c", [64, 1])
    w7T = din("w7T", [7, 64])
    b7c = din("b7c", [64, 1])
    L1T = din("L1T", [1152, 512])
    b8r = din("b8r", [1, 512])
    L2T = din("L2T", [512, 256])
    b9r = din("b9r", [1, 256])
    L3T = din("L3T", [256, 28])
    bL3r = din("bL3r", [1, 28])
    lvec = din("lvec", [5, 1])
    nvec = din("nvec", [7, 1])
    ident = din("ident", [128, 128])
    iota_pat = din("iota_pat", [128, N], u32)   # column index, replicated
    mask_col = din("mask_col", [128, 1], u32)   # 0xFFFFF800
    MA_d = din("MA", [32, 128])                 # selector: row == r%16
    MB_d = din("MB", [32, 128])                 # selector: row == 16 + r%16

    out_t = nc.dram_tensor("out", [1, 28], fp32, kind="ExternalOutput")

    groups = [[2 * i, 2 * i + 1] for i in range(max(1, n_cores // 2))]

    with tile.TileContext(nc) as tc:
        pid = nc.partition_id()
        off = (pid & 1) * HALF          # this core's first point column
        other_off = HALF - off          # the pair core's first point column
        other_rank = 1 - (pid & 1)

        # ---------------- pools ----------------
        consts = tc.alloc_tile_pool(name="consts", bufs=1)
        xcmp = tc.alloc_tile_pool(name="xcmp", bufs=1)
        dramp = tc.alloc_tile_pool(name="dramp", bufs=1, space="DRAM")
        lw = tc.alloc_tile_pool(name="lw", bufs=1)      # layer-wide tiles
        pB = tc.alloc_tile_pool(name="pB", bufs=2)      # big per-block tiles
        pS = tc.alloc_tile_pool(name="pS", bufs=4)      # small per-block tiles
        psA = tc.alloc_tile_pool(name="psA", bufs=2, space="PSUM")  # pd chunks
        psM = tc.alloc_tile_pool(name="psM", bufs=2, space="PSUM")  # [128,512]
        psS = tc.alloc_tile_pool(name="psS", bufs=2, space="PSUM")  # [128,128]

        # ---------------- persistent consts ----------------
        ident_sb = consts.tile([128, 128], fp32, name="ident_sb")
        nc.sync.dma_start(ident_sb[:], ident[:, :])
        iota_sb = consts.tile([128, N], u32, name="iota_sb")
        nc.sync.dma_start(iota_sb[:], iota_pat[:, :])
        mask_sb = consts.tile([128, 1], u32, name="mask_sb")
        nc.sync.dma_start(mask_sb[:], mask_col[:, :])
        MA_sb = consts.tile([32, 128], fp32, name="MA_sb")
        nc.sync.dma_start(MA_sb[:], MA_d[:, :])
        MB_sb = consts.tile([32, 128], fp32, name="MB_sb")
        nc.sync.dma_start(MB_sb[:], MB_d[:, :])
        ones_col = consts.tile([128, 1], fp32, name="ones_col")
        nc.vector.memset(ones_col[:], 1.0)
        ones_row = consts.tile([1, 128], fp32, name="ones_row")
        nc.vector.memset(ones_row[:], 1.0)
        ones_half = consts.tile([1, HALF], fp32, name="ones_half")
        nc.vector.memset(ones_half[:], 1.0)
        neg1_row = consts.tile([1, N], fp32, name="neg1_row")
        nc.vector.memset(neg1_row[:], -1.0)

        # weights: L4 split into two 128-column halves
        wy_sb, wz_sb = [], []
        for li, (c, o) in enumerate(LAYERS):
            zr = c + 2 if li < 3 else c  # wz rows: [(Wc-Wn); 0; b] for L1-3
            if o <= 128:
                t1 = consts.tile([c, o], fp32, name=f"wy_sb{li}")
                nc.sync.dma_start(t1[:], wy_d[li][:, :])
                wy_sb.append([t1])
                t2 = consts.tile([zr, o], fp32, name=f"wz_sb{li}")
                nc.sync.dma_start(t2[:], wz_d[li][:, :])
                wz_sb.append([t2])
            else:
                ys, zs = [], []
                for h in range(o // 128):
                    t1 = consts.tile([c, 128], fp32, name=f"wy_sb{li}_{h}")
                    nc.sync.dma_start(t1[:], wy_d[li][:, ts(h, 128)])
                    ys.append(t1)
                    t2 = consts.tile([zr, 128], fp32, name=f"wz_sb{li}_{h}")
                    nc.sync.dma_start(t2[:], wz_d[li][:, ts(h, 128)])
                    zs.append(t2)
                wy_sb.append(ys)
                wz_sb.append(zs)
        bz4_sb = [consts.tile([1, 128], fp32, name=f"bz4_sb{h}")
                  for h in range(2)]
        for h in range(2):
            nc.sync.dma_start(bz4_sb[h][:], bz_d[:, ts(h, 128)])

        # channel-major layer inputs (x_cm[1..3] double as x1..x3 for head)
        x_cm = [
            xcmp.tile([c, N], fp32, name=f"x_cm{li}")
            for li, (c, o) in enumerate(LAYERS)
        ]
        nc.sync.dma_start(x_cm[0][:], x0[:, :])
        # layer-4 output, my half, channel-major
        x4_my = [xcmp.tile([128, HALF], fp32, name=f"x4_my{j}") for j in range(2)]

        # ---------------- DRAM scratch ----------------
        xchg_in = [
            dramp.tile([o, HALF], fp32, name=f"xchg_in{li}")
            for li, (c, o) in enumerate(LAYERS[:3])
        ]
        xchg_out = [
            dramp.tile([2 * o, HALF], fp32, name=f"xchg_out{li}")
            for li, (c, o) in enumerate(LAYERS[:3])
        ]
        hred_in = dramp.tile([128, 8], fp32, name="hred_in")
        hred_out = dramp.tile([128, 8], fp32, name="hred_out")

        # ================= EdgeConv layers =================
        def edge_layer(li, c, o):
            last = li == 3
            xc = x_cm[li]
            nh = len(wy_sb[li])  # number of 128-wide output halves

            # ---- layer-wide: xx row, -xx bt row, yT, negxx_full ----
            xx_row = lw.tile([1, N], fp32, tag="xx_row")
            for q in range(4):
                xsq = lw.tile([c, 512], fp32, tag="xsq", bufs=2)
                nc.scalar.activation(xsq[:], xc[:, ts(q, 512)], AF.Square)
                mm = psM.tile([128, 512], fp32, tag="mm512")
                nc.tensor.matmul(
                    mm[0:1, :], ones_col[0:c, :], xsq[:], start=True, stop=True
                )
                nc.scalar.copy(xx_row[:, ts(q, 512)], mm[0:1, :])

            # bt = [2x; -1; -xx] for L1-3 (c+2 <= 128 rows, exact -d^2 with
            # a_my = [x; xx; 1]); L4: bt = 2x only, the -xx_j arrives via a
            # PSUM prefill and -xx_i via a rank-1 accumulate.
            negxx_row = lw.tile([1, N], fp32, tag="negxx_row")
            nc.scalar.activation(negxx_row[:], xx_row[:], AF.Copy, scale=-1.0)
            if c + 2 <= 128:
                bt = lw.tile([c + 2, N], fp32, tag="bt")
                bt_main_rows = c + 2
            else:
                bt = lw.tile([c, N], fp32, tag="bt")
                bt_main_rows = c
            for q in range(4):
                nc.scalar.activation(
                    bt[0:c, ts(q, 512)], xc[:, ts(q, 512)], AF.Copy, scale=2.0
                )
            if bt_main_rows == c + 2:
                nc.sync.dma_start(bt[c:c + 1, :], neg1_row[:])
                nc.sync.dma_start(bt[c + 1:c + 2, :], negxx_row[:])
                negxx_full = None
            else:
                # L4: prefill tile -xx broadcast to 128 partitions via PE
                negxx_full = lw.tile([128, N], fp32, tag="negxx_full")
                for q in range(4):
                    mm = psM.tile([128, 512], fp32, tag="mm512")
                    nc.tensor.matmul(
                        mm[:], ones_row[:], negxx_row[:, ts(q, 512)],
                        start=True, stop=True,
                    )
                    nc.scalar.copy(negxx_full[:, ts(q, 512)], mm[:])

            # yT [o, N] channel-major (nh tiles of <=128 partitions)
            yT = []
            for h in range(nh):
                ow = wy_sb[li][h].shape[1]
                t = lw.tile([ow, N], fp32, tag=f"yT{h}")
                for q in range(4):
                    mm = psM.tile([128, 512], fp32, tag="mm512")
                    nc.tensor.matmul(
                        mm[0:ow, :], wy_sb[li][h][:], xc[:, ts(q, 512)],
                        start=True, stop=True,
                    )
                    nc.scalar.copy(t[:, ts(q, 512)], mm[0:ow, :])
                yT.append(t)

            # pd lhsT: [x; xx; 1] (c+2 rows) for L1-3; L4 uses [x] plus a
            # separate xx_my row (rank-1 -xx_i accumulate).  Copied to
            # static-offset tiles: matmul operands reject register offsets.
            a_my = lw.tile([bt_main_rows, HALF], fp32, tag="a_my")
            nc.sync.dma_start(a_my[0:c, :], xc[:, ds(off, HALF)])
            if bt_main_rows == c + 2:
                nc.sync.dma_start(a_my[c:c + 1, :], xx_row[:, ds(off, HALF)])
                nc.sync.dma_start(a_my[c + 1:c + 2, :], ones_half[:])
                xx_my = None
            else:
                xx_my = lw.tile([1, HALF], fp32, tag="xx_my")
                nc.sync.dma_start(xx_my[:], xx_row[:, ds(off, HALF)])

            # EdgeConv output (my half, channel-major) at static offsets
            if not last:
                xo_my = [
                    lw.tile([t.shape[0], HALF], fp32, tag=f"xo_my{h}",
                            name=f"xo_my{li}_{h}")
                    for h, t in enumerate(yT)
                ]
            else:
                xo_my = x4_my

            # ---- per point-block of my half ----
            for i in range(NBLK):
                xsl = a_my[0:c, ts(i, 128)]  # [c, 128] static-offset slice

                # pd chunks -> packed [128, N] (fused mask|iota from PSUM)
                packed = pB.tile([128, N], fp32, tag="packed")
                for q in range(4):
                    pd_ps = psA.tile([128, 512], fp32, tag="pd_ps")
                    if negxx_full is not None:
                        nc.scalar.copy(pd_ps[:], negxx_full[:, ts(q, 512)])
                        nc.tensor.matmul(
                            pd_ps[:], xsl, bt[:, ts(q, 512)],
                            start=False, stop=False,
                        )
                        nc.tensor.matmul(
                            pd_ps[:], xx_my[:, ts(i, 128)],
                            neg1_row[:, 0:512], start=False, stop=True,
                        )
                    else:
                        nc.tensor.matmul(
                            pd_ps[:], a_my[:, ts(i, 128)],
                            bt[:, ts(q, 512)], start=True, stop=True,
                        )
                    nc.vector.scalar_tensor_tensor(
                        packed[:, ts(q, 512)].bitcast(u32),
                        pd_ps[:].bitcast(u32), mask_sb[:],
                        iota_sb[:, ts(q, 512)],
                        op0=OP.bitwise_and, op1=OP.bitwise_or,
                    )

                # chunk-pool top-32
                pool = pS.tile([128, 128], fp32, tag="pool")
                for ch in range(16):
                    nc.vector.max(
                        pool[:, ts(ch, 8)], packed[:, ts(ch, 128)]
                    )
                If_t = pS.tile([128, 32], fp32, tag="If_t")
                for r in range(4):
                    v8 = pS.tile([128, 8], fp32, tag="v8", bufs=8)
                    nc.vector.max(v8[:], pool[:])
                    pos8 = pS.tile([128, 8], u32, tag="pos8", bufs=8)
                    nc.vector.max_index(pos8[:], v8[:], pool[:])
                    # global idx = (pool_pos >> 3) * 128 | (packed & 0x7F)
                    pa = pS.tile([128, 8], u32, tag="pa", bufs=8)
                    nc.vector.tensor_scalar(
                        pa[:], v8[:].bitcast(u32), 127, None,
                        op0=OP.bitwise_and,
                    )
                    pb = pS.tile([128, 8], u32, tag="pb", bufs=8)
                    nc.vector.tensor_scalar(
                        pb[:], pos8[:], 3, 7,
                        op0=OP.logical_shift_right, op1=OP.logical_shift_left,
                    )
                    pc = pS.tile([128, 8], u32, tag="pc", bufs=8)
                    nc.vector.tensor_tensor(pc[:], pb[:], pa[:], op=OP.bitwise_or)
                    nc.vector.tensor_copy(If_t[:, ts(r, 8)], pc[:])
                    if r < 3:
                        nc.vector.match_replace(pool[:], v8[:], pool[:], NEG_BIG)

                # index re-layout for ap_gather
                it_ps = psS.tile([128, 128], fp32, tag="mm128")
                nc.tensor.transpose(it_ps[0:32, :], If_t[:, :], ident_sb[:])
                it_sb = pS.tile([32, 128], fp32, tag="it_sb")
                nc.scalar.copy(it_sb[:], it_ps[0:32, :])
                A_ps = psS.tile([128, 128], fp32, tag="mm128")
                nc.tensor.matmul(A_ps[:], MA_sb[:], it_sb[:], start=True, stop=True)
                B_ps = psS.tile([128, 128], fp32, tag="mm128")
                nc.tensor.matmul(B_ps[:], MB_sb[:], it_sb[:], start=True, stop=True)
                idxs_t = pS.tile([128, 128, 2], i16, tag="idxs_t")
                nc.vector.tensor_copy(idxs_t[:, :, 0:1], A_ps[:, :])
                nc.vector.tensor_copy(idxs_t[:, :, 1:2], B_ps[:, :])

                # gather + reduce + epilogue per output half
                for h in range(nh):
                    ow = yT[h].shape[0]
                    g = pB.tile([128, 128, K], fp32, tag="g")
                    nc.gpsimd.ap_gather(
                        g[0:ow, :, :], yT[h][:, :], idxs_t[0:ow, :, :],
                        channels=ow, num_elems=N, d=1, num_idxs=128 * K,
                    )
                    gmax = pS.tile([128, 128], fp32, tag="gmax", bufs=8)
                    nc.vector.tensor_reduce(
                        gmax[0:ow, :], g[0:ow, :, :], axis=AX.X, op=OP.max
                    )
                    # zT (+bias) and epilogue
                    z_ps = psS.tile([128, 128], fp32, tag="mm128")
                    if li < 3:
                        nc.tensor.matmul(
                            z_ps[0:ow, :], wz_sb[li][h][:],
                            a_my[:, ts(i, 128)], start=True, stop=True,
                        )
                    else:
                        nc.tensor.matmul(
                            z_ps[0:ow, :], wz_sb[li][h][:], xsl,
                            start=True, stop=False,
                        )
                        nc.tensor.matmul(
                            z_ps[0:ow, :], bz4_sb[h][:], ones_row[:],
                            start=False, stop=True,
                        )
                    u_t = pS.tile([128, 128], fp32, tag="u_t", bufs=8)
                    nc.vector.tensor_tensor(
                        u_t[0:ow, :], gmax[0:ow, :], z_ps[0:ow, :], op=OP.add
                    )
                    nc.vector.scalar_tensor_tensor(
                        xo_my[h][ds(0, ow), ts(i, 128)], u_t[0:ow, :], 0.2,
                        u_t[0:ow, :], op0=OP.mult, op1=OP.max,
                    )

            if not last:
                # my half into x_cm[li+1] and the exchange buffer
                for h, t in enumerate(xo_my):
                    ow = t.shape[0]
                    nc.sync.dma_start(
                        x_cm[li + 1][ds(h * 128, ow), ds(off, HALF)], t[:]
                    )
                    nc.sync.dma_start(xchg_in[li][ds(h * 128, ow), :], t[:])
                # ---- pair AllGather; fill the other half of x_cm[li+1] ----
                if n_cores == 1:
                    nc.sync.dma_start(
                        xchg_out[li][0:o, :], xchg_in[li][:, :]
                    )
                    nc.sync.dma_start(
                        xchg_out[li][o:2 * o, :], xchg_in[li][:, :]
                    )
                else:
                    nc.gpsimd.collective_compute(
                        "AllGather",
                        mybir.AluOpType.bypass,
                        replica_groups=groups,
                        ins=[xchg_in[li][:, :]],
                        outs=[xchg_out[li][:, :]],
                    )
                nc.sync.dma_start(
                    x_cm[li + 1][:, ds(other_off, HALF)],
                    xchg_out[li][ds(other_rank * o, o), :],
                )

        for li, (c, o) in enumerate(LAYERS):
            edge_layer(li, c, o)

        # ================= head =================
        psS.release()
        psM.release()
        psA.release()
        pS.release()
        pB.release()
        lw.release()
        w1 = tc.alloc_tile_pool(name="hw1", bufs=1)
        psA2 = tc.alloc_tile_pool(name="hpsA", bufs=1, space="PSUM")
        psC2 = tc.alloc_tile_pool(name="hpsC", bufs=1, space="PSUM")

        # my-half slices of x1..x3 at static offsets (matmul lhsT constraint)
        x1_my = w1.tile([64, HALF], fp32, tag="x1_my")
        nc.sync.dma_start(x1_my[:], x_cm[1][:, ds(off, HALF)])
        x2_my = w1.tile([64, HALF], fp32, tag="x2_my")
        nc.sync.dma_start(x2_my[:], x_cm[2][:, ds(off, HALF)])
        x3_my = w1.tile([128, HALF], fp32, tag="x3_my")
        nc.sync.dma_start(x3_my[:], x_cm[3][:, ds(off, HALF)])

        w5_sb = []
        for k2, (r0, r1) in enumerate([(0, 64), (64, 128), (128, 256),
                                       (256, 384), (384, 512)]):
            t = w1.tile([r1 - r0, 1024], fp32, tag=f"w5_{k2}")
            nc.sync.dma_start(t[:], w5[r0:r1, :])
            w5_sb.append(t)

        hmax = w1.tile([128, 1024], fp32, tag="hmax")
        for i in range(8):
            h_ps = psA2.tile([128, 1024], fp32, tag="h_ps")
            lhs = [x1_my[:, ts(i, 128)],
                   x2_my[:, ts(i, 128)],
                   x3_my[:, ts(i, 128)],
                   x4_my[0][:, ts(i, 128)],
                   x4_my[1][:, ts(i, 128)]]
            for q in range(2):
                for ci, l_ap in enumerate(lhs):
                    nc.tensor.matmul(
                        h_ps[:, ts(q, 512)], l_ap,
                        w5_sb[ci][:, ts(q, 512)],
                        start=(ci == 0), stop=(ci == len(lhs) - 1),
                    )
            if i == 0:
                nc.scalar.copy(hmax[:], h_ps[:])
            else:
                nc.vector.tensor_tensor(hmax[:], h_ps[:], hmax[:], op=OP.max)

        # partition-reduce via transposes -> [128, 8] (chan 128*j+p at [p, j])
        hcat = w1.tile([128, 8], fp32, tag="hcat")
        for j in range(8):
            tp = psC2.tile([128, 128], fp32, tag="tp")
            nc.tensor.transpose(tp[:], hmax[:, ts(j, 128)], ident_sb[:])
            nc.vector.tensor_reduce(
                hcat[:, j:j + 1], tp[:], axis=AX.X, op=OP.max
            )
        nc.sync.dma_start(hred_in[:, :], hcat[:])
        if n_cores == 1:
            nc.sync.dma_start(hred_out[:, :], hred_in[:, :])
        else:
            nc.gpsimd.collective_compute(
                "AllReduce", OP.max, replica_groups=groups,
                ins=[hred_in[:, :]], outs=[hred_out[:, :]],
            )
        hfull = w1.tile([128, 8], fp32, tag="hfull")
        nc.sync.dma_start(hfull[:], hred_out[:, :])
        b5_sb = w1.tile([128, 8], fp32, tag="b5_sb")
        nc.sync.dma_start(
            b5_sb[:], b5r.ap().rearrange("o (j p) -> (o p) j", p=128)
        )
        nc.vector.tensor_tensor(hfull[:], hfull[:], b5_sb[:], op=OP.add)
        nc.vector.scalar_tensor_tensor(
            hfull[:], hfull[:], 0.2, hfull[:], op0=OP.mult, op1=OP.max
        )

        # lf / nf columns
        lvec_sb = w1.tile([5, 1], fp32, tag="lvec_sb")
        nc.sync.dma_start(lvec_sb[:], lvec[:, :])
        nvec_sb = w1.tile([7, 1], fp32, tag="nvec_sb")
        nc.sync.dma_start(nvec_sb[:], nvec[:, :])
        w6_sb = w1.tile([5, 64], fp32, tag="w6_sb")
        nc.sync.dma_start(w6_sb[:], w6T[:, :])
        w7_sb = w1.tile([7, 64], fp32, tag="w7_sb")
        nc.sync.dma_start(w7_sb[:], w7T[:, :])
        b6_sb = w1.tile([64, 1], fp32, tag="b6_sb")
        nc.sync.dma_start(b6_sb[:], b6c[:, :])
        b7_sb = w1.tile([64, 1], fp32, tag="b7_sb")
        nc.sync.dma_start(b7_sb[:], b7c[:, :])

        def matvec_col(w_sb, v_sb, b_sb, n_out, tag):
            ps = psC2.tile([n_out, 1], fp32, tag="tpv")
            nc.tensor.matmul(ps[:], w_sb[:], v_sb[:], start=True, stop=True)
            r = w1.tile([n_out, 1], fp32, tag=tag)
            nc.vector.tensor_tensor(r[:], ps[:], b_sb[:], op=OP.add)
            nc.vector.scalar_tensor_tensor(
                r[:], r[:], 0.2, r[:], op0=OP.mult, op1=OP.max
            )
            return r

        lf_sb = matvec_col(w6_sb, lvec_sb, b6_sb, 64, "lf_sb")
        nf_sb = matvec_col(w7_sb, nvec_sb, b7_sb, 64, "nf_sb")

        # u tile [128, 9]: cols 0..7 = h, col 8 = [lf ; nf]
        u_t = w1.tile([128, 9], fp32, tag="u_t")
        nc.vector.tensor_copy(u_t[:, 0:8], hfull[:])
        nc.sync.dma_start(u_t[0:64, 8:9], lf_sb[:])
        nc.sync.dma_start(u_t[64:128, 8:9], nf_sb[:])

        def fc_row(v_cols, n_ch, wT_d, n_out, b_d, relu, tag):
            """out [1, n_out] = v.T @ wT ; v given as [128, n_ch] columns."""
            w_sb = w1.tile([128, n_ch, n_out], fp32, tag=f"{tag}_w")
            nc.sync.dma_start(
                w_sb[:], wT_d.ap().rearrange("(ch p) f -> p ch f", p=128)
            )
            ps = psC2.tile([1, n_out], fp32, tag="fcps")
            for ch in range(n_ch):
                nc.tensor.matmul(
                    ps[:], v_cols[:, ch:ch + 1], w_sb[:, ch, :],
                    start=(ch == 0), stop=(ch == n_ch - 1),
                )
            b_sb = w1.tile([1, n_out], fp32, tag=f"{tag}_b")
            nc.sync.dma_start(b_sb[:], b_d[:, :])
            r = w1.tile([1, n_out], fp32, tag=f"{tag}_r")
            nc.vector.tensor_tensor(r[:], ps[:], b_sb[:], op=OP.add)
            if relu:
                nc.vector.tensor_scalar_max(r[:], r[:], 0.0)
            return r

        def row_to_cols(v_row, n_ch, tag):
            """[1, 128*n_ch] -> [128, n_ch] via PE transposes."""
            cols = w1.tile([128, n_ch], fp32, tag=tag)
            for j in range(n_ch):
                tpv = psC2.tile([128, 1], fp32, tag="tpv2")
                nc.tensor.transpose(
                    tpv[:], v_row[:, ts(j, 128)], ident_sb[0:1, 0:1]
                )
                nc.vector.tensor_copy(cols[:, j:j + 1], tpv[:])
            return cols

        v1 = fc_row(u_t, 9, L1T, 512, b8r, True, "fc1")
        v1c = row_to_cols(v1, 4, "v1c")
        v2 = fc_row(v1c, 4, L2T, 256, b9r, True, "fc2")
        v2c = row_to_cols(v2, 2, "v2c")
        v3 = fc_row(v2c, 2, L3T, 28, bL3r, False, "fc3")
        nc.sync.dma_start(out_t[:, :], v3[:])

        for p in (psC2, psA2, w1, dramp, xcmp, consts):
            p.release()

    nc.compile()
    return nc


_PROGRAM_CACHE = {}


def get_program(n_cores=NCORES):
    key = n_cores
    if key not in _PROGRAM_CACHE:
        _PROGRAM_CACHE[key] = _build_program(n_cores)
    return _PROGRAM_CACHE[key]


def make_in_maps(inputs, n_cores=NCORES):
    """Host-side preprocessing: fold BN into weights, build per-core inputs."""
    f32 = np.float32

    def arr(v):
        return np.ascontiguousarray(np.asarray(v), dtype=f32)

    x = arr(inputs["x"])  # [B, 3, N]
    lmat = arr(inputs["l"])  # [B, 5]
    nmat = arr(inputs["n"])  # [B, 7]

    def fold(g):
        return arr(g) / np.sqrt(f32(1.0) + f32(EPS), dtype=f32)

    common = {}
    for li, (wn, gn, bn) in enumerate(
        [("W1", "g1", "b1"), ("W2", "g2", "b2"), ("W3", "g3", "b3"),
         ("W4", "g4", "b4")]
    ):
        W = arr(inputs[wn])  # [O, 2C]
        s = fold(inputs[gn])
        b = arr(inputs[bn])
        C = W.shape[1] // 2
        Wn = W[:, :C] * s[:, None]
        Wc = W[:, C:] * s[:, None]
        common[f"wy{li}"] = arr(Wn.T)
        wzT = arr((Wc - Wn).T)
        if li < 3:
            common[f"wz{li}"] = arr(np.concatenate(
                [wzT, np.zeros((1, len(b)), f32), b[None, :]], axis=0))
        else:
            common[f"wz{li}"] = wzT
            common["bz3"] = arr(b)[None, :]

    s5 = fold(inputs["g5"])
    common["w5"] = arr((arr(inputs["W5"]) * s5[:, None]).T)
    common["b5r"] = arr(inputs["b5"])[None, :]
    s6 = fold(inputs["g6"])
    common["w6T"] = arr((arr(inputs["W6"]) * s6[:, None]).T)
    common["b6c"] = arr(inputs["b6"])[:, None]
    s7 = fold(inputs["g7"])
    common["w7T"] = arr((arr(inputs["W7"]) * s7[:, None]).T)
    common["b7c"] = arr(inputs["b7"])[:, None]
    s8 = fold(inputs["g8"])
    common["L1T"] = arr((arr(inputs["L1"]) * s8[:, None]).T)
    common["b8r"] = arr(inputs["b8"])[None, :]
    s9 = fold(inputs["g9"])
    common["L2T"] = arr((arr(inputs["L2"]) * s9[:, None]).T)
    common["b9r"] = arr(s9 * arr(inputs["L2b"]) + arr(inputs["b9"]))[None, :]
    common["L3T"] = arr(arr(inputs["L3"]).T)
    common["bL3r"] = arr(inputs["L3b"])[None, :]
    common["ident"] = np.eye(128, dtype=f32)
    common["iota_pat"] = np.ascontiguousarray(
        np.broadcast_to(np.arange(N, dtype=np.uint32) % 128, (128, N))
    )
    common["mask_col"] = np.full((128, 1), IDX_MASK, dtype=np.uint32)
    MA = np.zeros((32, 128), f32)
    MB = np.zeros((32, 128), f32)
    for r in range(128):
        MA[r % 16, r] = 1.0
        MB[16 + r % 16, r] = 1.0
    common["MA"] = MA
    common["MB"] = MB

    in_maps = []
    for core in range(n_cores):
        b_i = (core // 2) % B
        m = dict(common)
        m["x0"] = arr(x[b_i])
        m["lvec"] = arr(lmat[b_i])[:, None]
        m["nvec"] = arr(nmat[b_i])[:, None]
        in_maps.append(m)
    return in_maps


LAST_RESULTS = None


def kernel(**inputs):
    global LAST_RESULTS
    from concourse.bass_utils import run_bass_kernel_spmd

    nc = get_program(NCORES)
    in_maps = make_in_maps(inputs, NCORES)
    res = run_bass_kernel_spmd(nc, in_maps, core_ids=list(range(NCORES)))
    LAST_RESULTS = res
    rows = [res.results[2 * b]["out"].reshape(28) for b in range(B)]
    return np.stack(rows, axis=0).astype(np.float32)


# revision 23
# speedup vs baseline: 1.0364x; 1.0011x over previous
"""DGCNN (4-layer EdgeConv + head) Bass kernel for 8 Trainium2 NeuronCores.

Problem: nn_DGCNN_net (B=4, N=2048, K=32), eval-mode BN.

Sharding: 2 cores per batch element (B=4 x 2-way split of the N=2048 points).
Cores 2b,2b+1 process batch b; even core owns points 0..1023, odd core
1024..2047. After each of the first three EdgeConv layers the pair exchanges
its half of the new features (pair-wise AllGather); the global max-pool is
combined with a pair-wise AllReduce(max).

v2 design (channel-major, SBUF gather):
  - Everything stays channel-major [chan, points]: y = (s*Wnbr)@x is computed
    as yT [o, N] directly, the neighbor gather runs on GPSIMD via ap_gather
    (SBUF->SBUF free-axis gather, one instruction per 128-point block instead
    of 31 descriptor-generating indirect DMAs), and the EdgeConv output comes
    out channel-major, which is exactly the next layer's input layout - no
    transpose rebuild.
  - k-NN selection: pd[i,j] = 2 x_i.x_j - xx_j (the -xx_i row constant is
    dropped; it does not change the row-wise top-k).  The column index is
    packed into the low 11 mantissa bits of the fp32 pd value
    (packed = (pd & 0xFFFFF800) | j, one fused scalar_tensor_tensor reading
    the PSUM matmul result).  For same-sign floats bit-pattern order ==
    value order, so top-k of packed == top-k of pd quantized to ~2^-12
    relative - and the selected values carry their indices for free.
  - top-32 per row: 16x Max8 over 128-wide chunks -> 128-entry pool, then
    4 rounds of Max8 + 3 MatchReplace on the pool only.  (Exact unless a
    single 128-chunk holds >8 of the row's true top-32: P ~ 7e-4 per row.)
  - The [128,32] u32 index tile is re-laid into ap_gather's wrapped int16
    format [16, 2p+h] with one PE transpose + two selector matmuls + two
    strided fp32->int16 copies.
"""

import numpy as np

EPS = 1e-5
K = 32
N = 2048
B = 4
NCORES = 8
HALF = N // 2
NBLK = HALF // 128  # 8 point-blocks per core

# layer configs: (C_in, O_out)
LAYERS = [(3, 64), (64, 64), (64, 128), (128, 256)]

NEG_BIG = -3.0e38
IDX_MASK = 0xFFFFFF80  # clear low 7 mantissa bits (local idx)


def _build_program(n_cores: int, reduce_on_pool=(False, False, True, True)):
    import concourse.bass as bass
    import concourse.mybir as mybir
    import concourse.bacc as bacc
    import concourse.tile as tile
    from concourse.bass import ds, ts

    fp32 = mybir.dt.float32
    bf16 = mybir.dt.bfloat16
    u32 = mybir.dt.uint32
    i16 = mybir.dt.int16
    AF = mybir.ActivationFunctionType
    OP = mybir.AluOpType
    AX = mybir.AxisListType

    nc = bacc.Bacc(
        "TRN2",
        target_bir_lowering=False,
        debug=False,
        num_devices=n_cores,
    )

    # ---------------- external IO ----------------
    def din(name, shape, dt=fp32):
        return nc.dram_tensor(name, shape, dt, kind="ExternalInput")

    x0 = din("x0", [3, N])
    # per layer: wy [c, o] (NO bias), wz [c, o], bz [o, 1]
    wy_d = [din(f"wy{li}", [c, o]) for li, (c, o) in enumerate(LAYERS)]
    wz_d = [din(f"wz{li}", [c + 2 if li < 3 else c, o])
            for li, (c, o) in enumerate(LAYERS)]
    bz_d = din("bz3", [1, 256])  # L4 bias row (others folded into wz)
    w5 = din("w5", [512, 1024])
    b5r = din("b5r", [1, 1024])
    w6T = din("w6T", [5, 64])
    b6c = din("b6c", [64, 1])
    w7T = din("w7T", [7, 64])
    b7c = din("b7c", [64, 1])
    L1T = din("L1T", [1152, 512])
    b8r = din("b8r", [1, 512])
    L2T = din("L2T", [512, 256])
    b9r = din("b9r", [1, 256])
    L3T = din("L3T", [256, 28])
    bL3r = din("bL3r", [1, 28])
    lvec = din("lvec", [5, 1])
    nvec = din("nvec", [7, 1])
    ident = din("ident", [128, 128])
    iota_pat = din("iota_pat", [128, N], u32)   # column index, replicated
    mask_col = din("mask_col", [128, 1], u32)   # 0xFFFFF800
    MA_d = din("MA", [32, 128])                 # selector: row == r%16
    MB_d = din("MB", [32, 128])                 # selector: row == 16 + r%16

    out_t = nc.dram_tensor("out", [1, 28], fp32, kind="ExternalOutput")

    groups = [[2 * i, 2 * i + 1] for i in range(max(1, n_cores // 2))]

    with tile.TileContext(nc) as tc:
        pid = nc.partition_id()
        off = (pid & 1) * HALF          # this core's first point column
        other_off = HALF - off          # the pair core's first point column
        other_rank = 1 - (pid & 1)

        # ---------------- pools ----------------
        consts = tc.alloc_tile_pool(name="consts", bufs=1)
        xcmp = tc.alloc_tile_pool(name="xcmp", bufs=1)
        dramp = tc.alloc_tile_pool(name="dramp", bufs=1, space="DRAM")
        lw = tc.alloc_tile_pool(name="lw", bufs=1)      # layer-wide tiles
        pB = tc.alloc_tile_pool(name="pB", bufs=2)      # big per-block tiles
        pS = tc.alloc_tile_pool(name="pS", bufs=4)      # small per-block tiles
        psA = tc.alloc_tile_pool(name="psA", bufs=2, space="PSUM")  # pd chunks
        psM = tc.alloc_tile_pool(name="psM", bufs=2, space="PSUM")  # [128,512]
        psS = tc.alloc_tile_pool(name="psS", bufs=2, space="PSUM")  # [128,128]

        # ---------------- persistent consts ----------------
        ident_sb = consts.tile([128, 128], fp32, name="ident_sb")
        nc.sync.dma_start(ident_sb[:], ident[:, :])
        iota_sb = consts.tile([128, N], u32, name="iota_sb")
        nc.sync.dma_start(iota_sb[:], iota_pat[:, :])
        mask_sb = consts.tile([128, 1], u32, name="mask_sb")
        nc.sync.dma_start(mask_sb[:], mask_col[:, :])
        MA_sb = consts.tile([32, 128], fp32, name="MA_sb")
        nc.sync.dma_start(MA_sb[:], MA_d[:, :])
        MB_sb = consts.tile([32, 128], fp32, name="MB_sb")
        nc.sync.dma_start(MB_sb[:], MB_d[:, :])
        ones_col = consts.tile([128, 1], fp32, name="ones_col")
        nc.vector.memset(ones_col[:], 1.0)
        ones_row = consts.tile([1, 128], fp32, name="ones_row")
        nc.vector.memset(ones_row[:], 1.0)
        ones_half = consts.tile([1, HALF], fp32, name="ones_half")
        nc.vector.memset(ones_half[:], 1.0)
        neg1_row = consts.tile([1, N], fp32, name="neg1_row")
        nc.vector.memset(neg1_row[:], -1.0)

        # weights: L4 split into two 128-column halves
        wy_sb, wz_sb = [], []
        for li, (c, o) in enumerate(LAYERS):
            zr = c + 2 if li < 3 else c  # wz rows: [(Wc-Wn); 0; b] for L1-3
            if o <= 128:
                t1 = consts.tile([c, o], fp32, name=f"wy_sb{li}")
                nc.sync.dma_start(t1[:], wy_d[li][:, :])
                wy_sb.append([t1])
                t2 = consts.tile([zr, o], fp32, name=f"wz_sb{li}")
                nc.sync.dma_start(t2[:], wz_d[li][:, :])
                wz_sb.append([t2])
            else:
                ys, zs = [], []
                for h in range(o // 128):
                    t1 = consts.tile([c, 128], fp32, name=f"wy_sb{li}_{h}")
                    nc.sync.dma_start(t1[:], wy_d[li][:, ts(h, 128)])
                    ys.append(t1)
                    t2 = consts.tile([zr, 128], fp32, name=f"wz_sb{li}_{h}")
                    nc.sync.dma_start(t2[:], wz_d[li][:, ts(h, 128)])
                    zs.append(t2)
                wy_sb.append(ys)
                wz_sb.append(zs)
        bz4_sb = [consts.tile([1, 128], fp32, name=f"bz4_sb{h}")
                  for h in range(2)]
        for h in range(2):
            nc.sync.dma_start(bz4_sb[h][:], bz_d[:, ts(h, 128)])

        # channel-major layer inputs (x_cm[1..3] double as x1..x3 for head)
        x_cm = [
            xcmp.tile([c, N], fp32, name=f"x_cm{li}")
            for li, (c, o) in enumerate(LAYERS)
        ]
        nc.sync.dma_start(x_cm[0][:], x0[:, :])
        # layer-4 output, my half, channel-major
        x4_my = [xcmp.tile([128, HALF], fp32, name=f"x4_my{j}") for j in range(2)]

        # ---------------- DRAM scratch ----------------
        xchg_in = [
            dramp.tile([o, HALF], fp32, name=f"xchg_in{li}")
            for li, (c, o) in enumerate(LAYERS[:3])
        ]
        xchg_out = [
            dramp.tile([2 * o, HALF], fp32, name=f"xchg_out{li}")
            for li, (c, o) in enumerate(LAYERS[:3])
        ]
        hred_in = dramp.tile([128, 8], fp32, name="hred_in")
        hred_out = dramp.tile([128, 8], fp32, name="hred_out")

        # ================= EdgeConv layers =================
        def edge_layer(li, c, o):
            last = li == 3
            xc = x_cm[li]
            nh = len(wy_sb[li])  # number of 128-wide output halves

            # ---- layer-wide: xx row, -xx bt row, yT, negxx_full ----
            xx_row = lw.tile([1, N], fp32, tag="xx_row")
            for q in range(4):
                xsq = lw.tile([c, 512], fp32, tag="xsq", bufs=2)
                nc.scalar.activation(xsq[:], xc[:, ts(q, 512)], AF.Square)
                mm = psM.tile([128, 512], fp32, tag="mm512")
                nc.tensor.matmul(
                    mm[0:1, :], ones_col[0:c, :], xsq[:], start=True, stop=True
                )
                nc.scalar.copy(xx_row[:, ts(q, 512)], mm[0:1, :])

            # bt = [2x; -1; -xx] for L1-3 (c+2 <= 128 rows, exact -d^2 with
            # a_my = [x; xx; 1]); L4: bt = 2x only, the -xx_j arrives via a
            # PSUM prefill and -xx_i via a rank-1 accumulate.
            negxx_row = lw.tile([1, N], fp32, tag="negxx_row")
            nc.scalar.activation(negxx_row[:], xx_row[:], AF.Copy, scale=-1.0)
            if c + 2 <= 128:
                bt = lw.tile([c + 2, N], fp32, tag="bt")
                bt_main_rows = c + 2
            else:
                bt = lw.tile([c, N], fp32, tag="bt")
                bt_main_rows = c
            for q in range(4):
                nc.scalar.activation(
                    bt[0:c, ts(q, 512)], xc[:, ts(q, 512)], AF.Copy, scale=2.0
                )
            if bt_main_rows == c + 2:
                nc.sync.dma_start(bt[c:c + 1, :], neg1_row[:])
                nc.sync.dma_start(bt[c + 1:c + 2, :], negxx_row[:])
                negxx_full = None
            else:
                # L4: prefill tile -xx broadcast to 128 partitions via PE
                negxx_full = lw.tile([128, N], fp32, tag="negxx_full")
                for q in range(4):
                    mm = psM.tile([128, 512], fp32, tag="mm512")
                    nc.tensor.matmul(
                        mm[:], ones_row[:], negxx_row[:, ts(q, 512)],
                        start=True, stop=True,
                    )
                    nc.scalar.copy(negxx_full[:, ts(q, 512)], mm[:])

            # yT [o, N] channel-major (nh tiles of <=128 partitions)
            yT = []
            for h in range(nh):
                ow = wy_sb[li][h].shape[1]
                t = lw.tile([ow, N], fp32, tag=f"yT{h}")
                for q in range(4):
                    mm = psM.tile([128, 512], fp32, tag="mm512")
                    nc.tensor.matmul(
                        mm[0:ow, :], wy_sb[li][h][:], xc[:, ts(q, 512)],
                        start=True, stop=True,
                    )
                    nc.scalar.copy(t[:, ts(q, 512)], mm[0:ow, :])
                yT.append(t)

            # pd lhsT: [x; xx; 1] (c+2 rows) for L1-3; L4 uses [x] plus a
            # separate xx_my row (rank-1 -xx_i accumulate).  Copied to
            # static-offset tiles: matmul operands reject register offsets.
            a_my = lw.tile([bt_main_rows, HALF], fp32, tag="a_my")
            nc.sync.dma_start(a_my[0:c, :], xc[:, ds(off, HALF)])
            if bt_main_rows == c + 2:
                nc.sync.dma_start(a_my[c:c + 1, :], xx_row[:, ds(off, HALF)])
                nc.sync.dma_start(a_my[c + 1:c + 2, :], ones_half[:])
                xx_my = None
            else:
                xx_my = lw.tile([1, HALF], fp32, tag="xx_my")
                nc.sync.dma_start(xx_my[:], xx_row[:, ds(off, HALF)])

            # EdgeConv output (my half, channel-major) at static offsets
            if not last:
                xo_my = [
                    lw.tile([t.shape[0], HALF], fp32, tag=f"xo_my{h}",
                            name=f"xo_my{li}_{h}")
                    for h, t in enumerate(yT)
                ]
            else:
                xo_my = x4_my

            # ---- per point-block of my half ----
            for i in range(NBLK):
                xsl = a_my[0:c, ts(i, 128)]  # [c, 128] static-offset slice

                # pd chunks -> packed [128, N] (fused mask|iota from PSUM)
                packed = pB.tile([128, N], fp32, tag="packed")
                for q in range(4):
                    pd_ps = psA.tile([128, 512], fp32, tag="pd_ps")
                    if negxx_full is not None:
                        nc.scalar.copy(pd_ps[:], negxx_full[:, ts(q, 512)])
                        nc.tensor.matmul(
                            pd_ps[:], xsl, bt[:, ts(q, 512)],
                            start=False, stop=True,
                        )
                    else:
                        nc.tensor.matmul(
                            pd_ps[:], a_my[:, ts(i, 128)],
                            bt[:, ts(q, 512)], start=True, stop=True,
                        )
                    nc.vector.scalar_tensor_tensor(
                        packed[:, ts(q, 512)].bitcast(u32),
                        pd_ps[:].bitcast(u32), mask_sb[:],
                        iota_sb[:, ts(q, 512)],
                        op0=OP.bitwise_and, op1=OP.bitwise_or,
                    )

                # chunk-pool top-32
                pool = pS.tile([128, 128], fp32, tag="pool")
                for ch in range(16):
                    nc.vector.max(
                        pool[:, ts(ch, 8)], packed[:, ts(ch, 128)]
                    )
                If_t = pS.tile([128, 32], fp32, tag="If_t")
                for r in range(4):
                    v8 = pS.tile([128, 8], fp32, tag="v8", bufs=8)
                    nc.vector.max(v8[:], pool[:])
                    pos8 = pS.tile([128, 8], u32, tag="pos8", bufs=8)
                    nc.vector.max_index(pos8[:], v8[:], pool[:])
                    # global idx = (pool_pos >> 3) * 128 | (packed & 0x7F)
                    pa = pS.tile([128, 8], u32, tag="pa", bufs=8)
                    nc.vector.tensor_scalar(
                        pa[:], v8[:].bitcast(u32), 127, None,
                        op0=OP.bitwise_and,
                    )
                    pb = pS.tile([128, 8], u32, tag="pb", bufs=8)
                    nc.vector.tensor_scalar(
                        pb[:], pos8[:], 3, 7,
                        op0=OP.logical_shift_right, op1=OP.logical_shift_left,
                    )
                    pc = pS.tile([128, 8], u32, tag="pc", bufs=8)
                    nc.vector.tensor_tensor(pc[:], pb[:], pa[:], op=OP.bitwise_or)
                    nc.vector.tensor_copy(If_t[:, ts(r, 8)], pc[:])
                    if r < 3:
                        nc.vector.match_replace(pool[:], v8[:], pool[:], NEG_BIG)

                # index re-layout for ap_gather
                it_ps = psS.tile([128, 128], fp32, tag="mm128")
                nc.tensor.transpose(it_ps[0:32, :], If_t[:, :], ident_sb[:])
                it_sb = pS.tile([32, 128], fp32, tag="it_sb")
                nc.scalar.copy(it_sb[:], it_ps[0:32, :])
                A_ps = psS.tile([128, 128], fp32, tag="mm128")
                nc.tensor.matmul(A_ps[:], MA_sb[:], it_sb[:], start=True, stop=True)
                B_ps = psS.tile([128, 128], fp32, tag="mm128")
                nc.tensor.matmul(B_ps[:], MB_sb[:], it_sb[:], start=True, stop=True)
                idxs_t = pS.tile([128, 128, 2], i16, tag="idxs_t")
                nc.vector.tensor_copy(idxs_t[:, :, 0:1], A_ps[:, :])
                nc.vector.tensor_copy(idxs_t[:, :, 1:2], B_ps[:, :])

                # gather + reduce + epilogue per output half
                for h in range(nh):
                    ow = yT[h].shape[0]
                    g = pB.tile([128, 128, K], fp32, tag="g")
                    nc.gpsimd.ap_gather(
                        g[0:ow, :, :], yT[h][:, :], idxs_t[0:ow, :, :],
                        channels=ow, num_elems=N, d=1, num_idxs=128 * K,
                    )
                    gmax = pS.tile([128, 128], fp32, tag="gmax", bufs=8)
                    nc.vector.tensor_reduce(
                        gmax[0:ow, :], g[0:ow, :, :], axis=AX.X, op=OP.max
                    )
                    # zT (+bias) and epilogue
                    z_ps = psS.tile([128, 128], fp32, tag="mm128")
                    if li < 3:
                        nc.tensor.matmul(
                            z_ps[0:ow, :], wz_sb[li][h][:],
                            a_my[:, ts(i, 128)], start=True, stop=True,
                        )
                    else:
                        nc.tensor.matmul(
                            z_ps[0:ow, :], wz_sb[li][h][:], xsl,
                            start=True, stop=False,
                        )
                        nc.tensor.matmul(
                            z_ps[0:ow, :], bz4_sb[h][:], ones_row[:],
                            start=False, stop=True,
                        )
                    u_t = pS.tile([128, 128], fp32, tag="u_t", bufs=8)
                    nc.vector.tensor_tensor(
                        u_t[0:ow, :], gmax[0:ow, :], z_ps[0:ow, :], op=OP.add
                    )
                    nc.vector.scalar_tensor_tensor(
                        xo_my[h][ds(0, ow), ts(i, 128)], u_t[0:ow, :], 0.2,
                        u_t[0:ow, :], op0=OP.mult, op1=OP.max,
                    )

            if not last:
                # my half into x_cm[li+1] and the exchange buffer
                for h, t in enumerate(xo_my):
                    ow = t.shape[0]
                    nc.sync.dma_start(
                        x_cm[li + 1][ds(h * 128, ow), ds(off, HALF)], t[:]
                    )
                    nc.sync.dma_start(xchg_in[li][ds(h * 128, ow), :], t[:])
                # ---- pair AllGather; fill the other half of x_cm[li+1] ----
                if n_cores == 1:
                    nc.sync.dma_start(
                        xchg_out[li][0:o, :], xchg_in[li][:, :]
                    )
                    nc.sync.dma_start(
                        xchg_out[li][o:2 * o, :], xchg_in[li][:, :]
                    )
                else:
                    nc.gpsimd.collective_compute(
                        "AllGather",
                        mybir.AluOpType.bypass,
                        replica_groups=groups,
                        ins=[xchg_in[li][:, :]],
                        outs=[xchg_out[li][:, :]],
                    )
                nc.sync.dma_start(
                    x_cm[li + 1][:, ds(other_off, HALF)],
                    xchg_out[li][ds(other_rank * o, o), :],
                )

        for li, (c, o) in enumerate(LAYERS):
            edge_layer(li, c, o)

        # ================= head =================
        psS.release()
        psM.release()
        psA.release()
        pS.release()
        pB.release()
        lw.release()
        w1 = tc.alloc_tile_pool(name="hw1", bufs=1)
        psA2 = tc.alloc_tile_pool(name="hpsA", bufs=1, space="PSUM")
        psC2 = tc.alloc_tile_pool(name="hpsC", bufs=1, space="PSUM")

        # my-half slices of x1..x3 at static offsets (matmul lhsT constraint)
        x1_my = w1.tile([64, HALF], fp32, tag="x1_my")
        nc.sync.dma_start(x1_my[:], x_cm[1][:, ds(off, HALF)])
        x2_my = w1.tile([64, HALF], fp32, tag="x2_my")
        nc.sync.dma_start(x2_my[:], x_cm[2][:, ds(off, HALF)])
        x3_my = w1.tile([128, HALF], fp32, tag="x3_my")
        nc.sync.dma_start(x3_my[:], x_cm[3][:, ds(off, HALF)])

        w5_sb = []
        for k2, (r0, r1) in enumerate([(0, 64), (64, 128), (128, 256),
                                       (256, 384), (384, 512)]):
            t = w1.tile([r1 - r0, 1024], fp32, tag=f"w5_{k2}")
            nc.sync.dma_start(t[:], w5[r0:r1, :])
            w5_sb.append(t)

        hmax = w1.tile([128, 1024], fp32, tag="hmax")
        for i in range(8):
            h_ps = psA2.tile([128, 1024], fp32, tag="h_ps")
            lhs = [x1_my[:, ts(i, 128)],
                   x2_my[:, ts(i, 128)],
                   x3_my[:, ts(i, 128)],
                   x4_my[0][:, ts(i, 128)],
                   x4_my[1][:, ts(i, 128)]]
            for q in range(2):
                for ci, l_ap in enumerate(lhs):
                    nc.tensor.matmul(
                        h_ps[:, ts(q, 512)], l_ap,
                        w5_sb[ci][:, ts(q, 512)],
                        start=(ci == 0), stop=(ci == len(lhs) - 1),
                    )
            if i == 0:
                nc.scalar.copy(hmax[:], h_ps[:])
            else:
                nc.vector.tensor_tensor(hmax[:], h_ps[:], hmax[:], op=OP.max)

        # partition-reduce via transposes -> [128, 8] (chan 128*j+p at [p, j])
        hcat = w1.tile([128, 8], fp32, tag="hcat")
        for j in range(8):
            tp = psC2.tile([128, 128], fp32, tag="tp")
            nc.tensor.transpose(tp[:], hmax[:, ts(j, 128)], ident_sb[:])
            nc.vector.tensor_reduce(
                hcat[:, j:j + 1], tp[:], axis=AX.X, op=OP.max
            )
        nc.sync.dma_start(hred_in[:, :], hcat[:])
        if n_cores == 1:
            nc.sync.dma_start(hred_out[:, :], hred_in[:, :])
        else:
            nc.gpsimd.collective_compute(
                "AllReduce", OP.max, replica_groups=groups,
                ins=[hred_in[:, :]], outs=[hred_out[:, :]],
            )
        hfull = w1.tile([128, 8], fp32, tag="hfull")
        nc.sync.dma_start(hfull[:], hred_out[:, :])
        b5_sb = w1.tile([128, 8], fp32, tag="b5_sb")
        nc.sync.dma_start(
            b5_sb[:], b5r.ap().rearrange("o (j p) -> (o p) j", p=128)
        )
        nc.vector.tensor_tensor(hfull[:], hfull[:], b5_sb[:], op=OP.add)
        nc.vector.scalar_tensor_tensor(
            hfull[:], hfull[:], 0.2, hfull[:], op0=OP.mult, op1=OP.max
        )

        # lf / nf columns
        lvec_sb = w1.tile([5, 1], fp32, tag="lvec_sb")
        nc.sync.dma_start(lvec_sb[:], lvec[:, :])
        nvec_sb = w1.tile([7, 1], fp32, tag="nvec_sb")
        nc.sync.dma_start(nvec_sb[:], nvec[:, :])
        w6_sb = w1.tile([5, 64], fp32, tag="w6_sb")
        nc.sync.dma_start(w6_sb[:], w6T[:, :])
        w7_sb = w1.tile([7, 64], fp32, tag="w7_sb")
        nc.sync.dma_start(w7_sb[:], w7T[:, :])
        b6_sb = w1.tile([64, 1], fp32, tag="b6_sb")
        nc.sync.dma_start(b6_sb[:], b6c[:, :])
        b7_sb = w1.tile([64, 1], fp32, tag="b7_sb")
        nc.sync.dma_start(b7_sb[:], b7c[:, :])

        def matvec_col(w_sb, v_sb, b_sb, n_out, tag):
            ps = psC2.tile([n_out, 1], fp32, tag="tpv")
            nc.tensor.matmul(ps[:], w_sb[:], v_sb[:], start=True, stop=True)
            r = w1.tile([n_out, 1], fp32, tag=tag)
            nc.vector.tensor_tensor(r[:], ps[:], b_sb[:], op=OP.add)
            nc.vector.scalar_tensor_tensor(
                r[:], r[:], 0.2, r[:], op0=OP.mult, op1=OP.max
            )
            return r

        lf_sb = matvec_col(w6_sb, lvec_sb, b6_sb, 64, "lf_sb")
        nf_sb = matvec_col(w7_sb, nvec_sb, b7_sb, 64, "nf_sb")

        # u tile [128, 9]: cols 0..7 = h, col 8 = [lf ; nf]
        u_t = w1.tile([128, 9], fp32, tag="u_t")
        nc.vector.tensor_copy(u_t[:, 0:8], hfull[:])
        nc.sync.dma_start(u_t[0:64, 8:9], lf_sb[:])
        nc.sync.dma_start(u_t[64:128, 8:9], nf_sb[:])

        def fc_row(v_cols, n_ch, wT_d, n_out, b_d, relu, tag):
            """out [1, n_out] = v.T @ wT ; v given as [128, n_ch] columns."""
            w_sb = w1.tile([128, n_ch, n_out], fp32, tag=f"{tag}_w")
            nc.sync.dma_start(
                w_sb[:], wT_d.ap().rearrange("(ch p) f -> p ch f", p=128)
            )
            ps = psC2.tile([1, n_out], fp32, tag="fcps")
            for ch in range(n_ch):
                nc.tensor.matmul(
                    ps[:], v_cols[:, ch:ch + 1], w_sb[:, ch, :],
                    start=(ch == 0), stop=(ch == n_ch - 1),
                )
            b_sb = w1.tile([1, n_out], fp32, tag=f"{tag}_b")
            nc.sync.dma_start(b_sb[:], b_d[:, :])
            r = w1.tile([1, n_out], fp32, tag=f"{tag}_r")
            nc.vector.tensor_tensor(r[:], ps[:], b_sb[:], op=OP.add)
            if relu:
                nc.vector.tensor_scalar_max(r[:], r[:], 0.0)
            return r

        def row_to_cols(v_row, n_ch, tag):
            """[1, 128*n_ch] -> [128, n_ch] via PE transposes."""
            cols = w1.tile([128, n_ch], fp32, tag=tag)
            for j in range(n_ch):
                tpv = psC2.tile([128, 1], fp32, tag="tpv2")
                nc.tensor.transpose(
                    tpv[:], v_row[:, ts(j, 128)], ident_sb[0:1, 0:1]
                )
                nc.vector.tensor_copy(cols[:, j:j + 1], tpv[:])
            return cols

        v1 = fc_row(u_t, 9, L1T, 512, b8r, True, "fc1")
        v1c = row_to_cols(v1, 4, "v1c")
        v2 = fc_row(v1c, 4, L2T, 256, b9r, True, "fc2")
        v2c = row_to_cols(v2, 2, "v2c")
        v3 = fc_row(v2c, 2, L3T, 28, bL3r, False, "fc3")
        nc.sync.dma_start(out_t[:, :], v3[:])

        for p in (psC2, psA2, w1, dramp, xcmp, consts):
            p.release()

    nc.compile()
    return nc


_PROGRAM_CACHE = {}


def get_program(n_cores=NCORES):
    key = n_cores
    if key not in _PROGRAM_CACHE:
        _PROGRAM_CACHE[key] = _build_program(n_cores)
    return _PROGRAM_CACHE[key]


def make_in_maps(inputs, n_cores=NCORES):
    """Host-side preprocessing: fold BN into weights, build per-core inputs."""
    f32 = np.float32

    def arr(v):
        return np.ascontiguousarray(np.asarray(v), dtype=f32)

    x = arr(inputs["x"])  # [B, 3, N]
    lmat = arr(inputs["l"])  # [B, 5]
    nmat = arr(inputs["n"])  # [B, 7]

    def fold(g):
        return arr(g) / np.sqrt(f32(1.0) + f32(EPS), dtype=f32)

    common = {}
    for li, (wn, gn, bn) in enumerate(
        [("W1", "g1", "b1"), ("W2", "g2", "b2"), ("W3", "g3", "b3"),
         ("W4", "g4", "b4")]
    ):
        W = arr(inputs[wn])  # [O, 2C]
        s = fold(inputs[gn])
        b = arr(inputs[bn])
        C = W.shape[1] // 2
        Wn = W[:, :C] * s[:, None]
        Wc = W[:, C:] * s[:, None]
        common[f"wy{li}"] = arr(Wn.T)
        wzT = arr((Wc - Wn).T)
        if li < 3:
            common[f"wz{li}"] = arr(np.concatenate(
                [wzT, np.zeros((1, len(b)), f32), b[None, :]], axis=0))
        else:
            common[f"wz{li}"] = wzT
            common["bz3"] = arr(b)[None, :]

    s5 = fold(inputs["g5"])
    common["w5"] = arr((arr(inputs["W5"]) * s5[:, None]).T)
    common["b5r"] = arr(inputs["b5"])[None, :]
    s6 = fold(inputs["g6"])
    common["w6T"] = arr((arr(inputs["W6"]) * s6[:, None]).T)
    common["b6c"] = arr(inputs["b6"])[:, None]
    s7 = fold(inputs["g7"])
    common["w7T"] = arr((arr(inputs["W7"]) * s7[:, None]).T)
    common["b7c"] = arr(inputs["b7"])[:, None]
    s8 = fold(inputs["g8"])
    common["L1T"] = arr((arr(inputs["L1"]) * s8[:, None]).T)
    common["b8r"] = arr(inputs["b8"])[None, :]
    s9 = fold(inputs["g9"])
    common["L2T"] = arr((arr(inputs["L2"]) * s9[:, None]).T)
    common["b9r"] = arr(s9 * arr(inputs["L2b"]) + arr(inputs["b9"]))[None, :]
    common["L3T"] = arr(arr(inputs["L3"]).T)
    common["bL3r"] = arr(inputs["L3b"])[None, :]
    common["ident"] = np.eye(128, dtype=f32)
    common["iota_pat"] = np.ascontiguousarray(
        np.broadcast_to(np.arange(N, dtype=np.uint32) % 128, (128, N))
    )
    common["mask_col"] = np.full((128, 1), IDX_MASK, dtype=np.uint32)
    MA = np.zeros((32, 128), f32)
    MB = np.zeros((32, 128), f32)
    for r in range(128):
        MA[r % 16, r] = 1.0
        MB[16 + r % 16, r] = 1.0
    common["MA"] = MA
    common["MB"] = MB

    in_maps = []
    for core in range(n_cores):
        b_i = (core // 2) % B
        m = dict(common)
        m["x0"] = arr(x[b_i])
        m["lvec"] = arr(lmat[b_i])[:, None]
        m["nvec"] = arr(nmat[b_i])[:, None]
        in_maps.append(m)
    return in_maps


LAST_RESULTS = None


def kernel(**inputs):
    global LAST_RESULTS
    from concourse.bass_utils import run_bass_kernel_spmd

    nc = get_program(NCORES)
    in_maps = make_in_maps(inputs, NCORES)
    res = run_bass_kernel_spmd(nc, in_maps, core_ids=list(range(NCORES)))
    LAST_RESULTS = res
    rows = [res.results[2 * b]["out"].reshape(28) for b in range(B)]
    return np.stack(rows, axis=0).astype(np.float32)


# revision 25
# speedup vs baseline: 1.1249x; 1.0855x over previous
"""DGCNN (4-layer EdgeConv + head) Bass kernel for 8 Trainium2 NeuronCores.

Problem: nn_DGCNN_net (B=4, N=2048, K=32), eval-mode BN.

Sharding: 2 cores per batch element (B=4 x 2-way split of the N=2048 points).
Cores 2b,2b+1 process batch b; even core owns points 0..1023, odd core
1024..2047. After each of the first three EdgeConv layers the pair exchanges
its half of the new features (pair-wise AllGather); the global max-pool is
combined with a pair-wise AllReduce(max).

v2 design (channel-major, SBUF gather):
  - Everything stays channel-major [chan, points]: y = (s*Wnbr)@x is computed
    as yT [o, N] directly, the neighbor gather runs on GPSIMD via ap_gather
    (SBUF->SBUF free-axis gather, one instruction per 128-point block instead
    of 31 descriptor-generating indirect DMAs), and the EdgeConv output comes
    out channel-major, which is exactly the next layer's input layout - no
    transpose rebuild.
  - k-NN selection: pd[i,j] = 2 x_i.x_j - xx_j (the -xx_i row constant is
    dropped; it does not change the row-wise top-k).  The column index is
    packed into the low 11 mantissa bits of the fp32 pd value
    (packed = (pd & 0xFFFFF800) | j, one fused scalar_tensor_tensor reading
    the PSUM matmul result).  For same-sign floats bit-pattern order ==
    value order, so top-k of packed == top-k of pd quantized to ~2^-12
    relative - and the selected values carry their indices for free.
  - top-32 per row: 16x Max8 over 128-wide chunks -> 128-entry pool, then
    4 rounds of Max8 + 3 MatchReplace on the pool only.  (Exact unless a
    single 128-chunk holds >8 of the row's true top-32: P ~ 7e-4 per row.)
  - The [128,32] u32 index tile is re-laid into ap_gather's wrapped int16
    format [16, 2p+h] with one PE transpose + two selector matmuls + two
    strided fp32->int16 copies.
"""

import numpy as np

EPS = 1e-5
K = 32
N = 2048
B = 4
NCORES = 8
HALF = N // 2
NBLK = HALF // 128  # 8 point-blocks per core

# layer configs: (C_in, O_out)
LAYERS = [(3, 64), (64, 64), (64, 128), (128, 256)]

NEG_BIG = -3.0e38
IDX_MASK = 0xFFFFFF80  # clear low 7 mantissa bits (local idx)


def _build_program(n_cores: int, reduce_on_pool=(False, False, True, True)):
    import concourse.bass as bass
    import concourse.mybir as mybir
    import concourse.bacc as bacc
    import concourse.tile as tile
    from concourse.bass import ds, ts

    fp32 = mybir.dt.float32
    bf16 = mybir.dt.bfloat16
    u32 = mybir.dt.uint32
    i16 = mybir.dt.int16
    AF = mybir.ActivationFunctionType
    OP = mybir.AluOpType
    AX = mybir.AxisListType

    nc = bacc.Bacc(
        "TRN2",
        target_bir_lowering=False,
        debug=False,
        num_devices=n_cores,
    )

    # ---------------- external IO ----------------
    def din(name, shape, dt=fp32):
        return nc.dram_tensor(name, shape, dt, kind="ExternalInput")

    x0 = din("x0", [3, N])
    # per layer: wy [c, o] (NO bias), wz [c, o], bz [o, 1]
    wy_d = [din(f"wy{li}", [c, o]) for li, (c, o) in enumerate(LAYERS)]
    wz_d = [din(f"wz{li}", [c + 2 if li < 3 else c, o])
            for li, (c, o) in enumerate(LAYERS)]
    bz_d = din("bz3", [1, 256])  # L4 bias row (others folded into wz)
    w5b = din("w5b", [512, 1024], mybir.dt.bfloat16)
    b5r = din("b5r", [1, 1024])
    w6T = din("w6T", [5, 64])
    b6c = din("b6c", [64, 1])
    w7T = din("w7T", [7, 64])
    b7c = din("b7c", [64, 1])
    L1T = din("L1T", [1152, 512])
    b8r = din("b8r", [1, 512])
    L2T = din("L2T", [512, 256])
    b9r = din("b9r", [1, 256])
    L3T = din("L3T", [256, 28])
    bL3r = din("bL3r", [1, 28])
    lvec = din("lvec", [5, 1])
    nvec = din("nvec", [7, 1])
    ident = din("ident", [128, 128])
    iota_pat = din("iota_pat", [128, N], u32)   # column index, replicated
    mask_col = din("mask_col", [128, 1], u32)   # 0xFFFFF800
    MA_d = din("MA", [32, 128])                 # selector: row == r%16
    MB_d = din("MB", [32, 128])                 # selector: row == 16 + r%16

    out_t = nc.dram_tensor("out", [1, 28], fp32, kind="ExternalOutput")

    groups = [[2 * i, 2 * i + 1] for i in range(max(1, n_cores // 2))]

    with tile.TileContext(nc) as tc:
        pid = nc.partition_id()
        off = (pid & 1) * HALF          # this core's first point column
        other_off = HALF - off          # the pair core's first point column
        other_rank = 1 - (pid & 1)

        # ---------------- pools ----------------
        consts = tc.alloc_tile_pool(name="consts", bufs=1)
        xcmp = tc.alloc_tile_pool(name="xcmp", bufs=1)
        dramp = tc.alloc_tile_pool(name="dramp", bufs=1, space="DRAM")
        lw = tc.alloc_tile_pool(name="lw", bufs=1)      # layer-wide tiles
        pB = tc.alloc_tile_pool(name="pB", bufs=2)      # big per-block tiles
        pS = tc.alloc_tile_pool(name="pS", bufs=4)      # small per-block tiles
        psA = tc.alloc_tile_pool(name="psA", bufs=2, space="PSUM")  # pd chunks
        psM = tc.alloc_tile_pool(name="psM", bufs=2, space="PSUM")  # [128,512]
        psS = tc.alloc_tile_pool(name="psS", bufs=2, space="PSUM")  # [128,128]

        # ---------------- persistent consts ----------------
        ident_sb = consts.tile([128, 128], fp32, name="ident_sb")
        nc.sync.dma_start(ident_sb[:], ident[:, :])
        iota_sb = consts.tile([128, N], u32, name="iota_sb")
        nc.sync.dma_start(iota_sb[:], iota_pat[:, :])
        mask_sb = consts.tile([128, 1], u32, name="mask_sb")
        nc.sync.dma_start(mask_sb[:], mask_col[:, :])
        MA_sb = consts.tile([32, 128], fp32, name="MA_sb")
        nc.sync.dma_start(MA_sb[:], MA_d[:, :])
        MB_sb = consts.tile([32, 128], fp32, name="MB_sb")
        nc.sync.dma_start(MB_sb[:], MB_d[:, :])
        ones_col = consts.tile([128, 1], fp32, name="ones_col")
        nc.vector.memset(ones_col[:], 1.0)
        ones_row = consts.tile([1, 128], fp32, name="ones_row")
        nc.vector.memset(ones_row[:], 1.0)
        ones_half = consts.tile([1, HALF], fp32, name="ones_half")
        nc.vector.memset(ones_half[:], 1.0)
        neg1_row = consts.tile([1, N], fp32, name="neg1_row")
        nc.vector.memset(neg1_row[:], -1.0)

        # weights: L4 split into two 128-column halves
        wy_sb, wz_sb = [], []
        for li, (c, o) in enumerate(LAYERS):
            zr = c + 2 if li < 3 else c  # wz rows: [(Wc-Wn); 0; b] for L1-3
            if o <= 128:
                t1 = consts.tile([c, o], fp32, name=f"wy_sb{li}")
                nc.sync.dma_start(t1[:], wy_d[li][:, :])
                wy_sb.append([t1])
                t2 = consts.tile([zr, o], fp32, name=f"wz_sb{li}")
                nc.sync.dma_start(t2[:], wz_d[li][:, :])
                wz_sb.append([t2])
            else:
                ys, zs = [], []
                for h in range(o // 128):
                    t1 = consts.tile([c, 128], fp32, name=f"wy_sb{li}_{h}")
                    nc.sync.dma_start(t1[:], wy_d[li][:, ts(h, 128)])
                    ys.append(t1)
                    t2 = consts.tile([zr, 128], fp32, name=f"wz_sb{li}_{h}")
                    nc.sync.dma_start(t2[:], wz_d[li][:, ts(h, 128)])
                    zs.append(t2)
                wy_sb.append(ys)
                wz_sb.append(zs)
        bz4_sb = [consts.tile([1, 128], fp32, name=f"bz4_sb{h}")
                  for h in range(2)]
        for h in range(2):
            nc.sync.dma_start(bz4_sb[h][:], bz_d[:, ts(h, 128)])

        # channel-major layer inputs (x_cm[1..3] double as x1..x3 for head)
        x_cm = [
            xcmp.tile([c, N], fp32, name=f"x_cm{li}")
            for li, (c, o) in enumerate(LAYERS)
        ]
        nc.sync.dma_start(x_cm[0][:], x0[:, :])
        # layer-4 output, my half, channel-major
        x4_my = [xcmp.tile([128, HALF], fp32, name=f"x4_my{j}") for j in range(2)]

        # ---------------- DRAM scratch ----------------
        xchg_in = [
            dramp.tile([o, HALF], fp32, name=f"xchg_in{li}")
            for li, (c, o) in enumerate(LAYERS[:3])
        ]
        xchg_out = [
            dramp.tile([2 * o, HALF], fp32, name=f"xchg_out{li}")
            for li, (c, o) in enumerate(LAYERS[:3])
        ]
        hred_in = dramp.tile([128, 8], fp32, name="hred_in")
        hred_out = dramp.tile([128, 8], fp32, name="hred_out")

        # ================= EdgeConv layers =================
        def edge_layer(li, c, o):
            last = li == 3
            xc = x_cm[li]
            nh = len(wy_sb[li])  # number of 128-wide output halves

            # ---- layer-wide: xx row, -xx bt row, yT, negxx_full ----
            xx_row = lw.tile([1, N], fp32, tag="xx_row")
            for q in range(4):
                xsq = lw.tile([c, 512], fp32, tag="xsq", bufs=2)
                nc.scalar.activation(xsq[:], xc[:, ts(q, 512)], AF.Square)
                mm = psM.tile([128, 512], fp32, tag="mm512")
                nc.tensor.matmul(
                    mm[0:1, :], ones_col[0:c, :], xsq[:], start=True, stop=True
                )
                nc.scalar.copy(xx_row[:, ts(q, 512)], mm[0:1, :])

            # bt = [2x; -1; -xx] for L1-3 (c+2 <= 128 rows, exact -d^2 with
            # a_my = [x; xx; 1]); L4: bt = 2x only, the -xx_j arrives via a
            # PSUM prefill and -xx_i via a rank-1 accumulate.
            negxx_row = lw.tile([1, N], fp32, tag="negxx_row")
            nc.scalar.activation(negxx_row[:], xx_row[:], AF.Copy, scale=-1.0)
            if c + 2 <= 128:
                bt = lw.tile([c + 2, N], fp32, tag="bt")
                bt_main_rows = c + 2
            else:
                bt = lw.tile([c, N], fp32, tag="bt")
                bt_main_rows = c
            for q in range(4):
                nc.scalar.activation(
                    bt[0:c, ts(q, 512)], xc[:, ts(q, 512)], AF.Copy, scale=2.0
                )
            if bt_main_rows == c + 2:
                nc.sync.dma_start(bt[c:c + 1, :], neg1_row[:])
                nc.sync.dma_start(bt[c + 1:c + 2, :], negxx_row[:])
                negxx_full = None
            else:
                # L4: prefill tile -xx broadcast to 128 partitions via PE
                negxx_full = lw.tile([128, N], fp32, tag="negxx_full")
                for q in range(4):
                    mm = psM.tile([128, 512], fp32, tag="mm512")
                    nc.tensor.matmul(
                        mm[:], ones_row[:], negxx_row[:, ts(q, 512)],
                        start=True, stop=True,
                    )
                    nc.scalar.copy(negxx_full[:, ts(q, 512)], mm[:])

            # yT [o, N] channel-major (nh tiles of <=128 partitions)
            yT = []
            for h in range(nh):
                ow = wy_sb[li][h].shape[1]
                t = lw.tile([ow, N], fp32, tag=f"yT{h}")
                for q in range(4):
                    mm = psM.tile([128, 512], fp32, tag="mm512")
                    nc.tensor.matmul(
                        mm[0:ow, :], wy_sb[li][h][:], xc[:, ts(q, 512)],
                        start=True, stop=True,
                    )
                    nc.scalar.copy(t[:, ts(q, 512)], mm[0:ow, :])
                yT.append(t)

            # pd lhsT: [x; xx; 1] (c+2 rows) for L1-3; L4 uses [x] plus a
            # separate xx_my row (rank-1 -xx_i accumulate).  Copied to
            # static-offset tiles: matmul operands reject register offsets.
            a_my = lw.tile([bt_main_rows, HALF], fp32, tag="a_my")
            nc.sync.dma_start(a_my[0:c, :], xc[:, ds(off, HALF)])
            if bt_main_rows == c + 2:
                nc.sync.dma_start(a_my[c:c + 1, :], xx_row[:, ds(off, HALF)])
                nc.sync.dma_start(a_my[c + 1:c + 2, :], ones_half[:])
                xx_my = None
            else:
                xx_my = lw.tile([1, HALF], fp32, tag="xx_my")
                nc.sync.dma_start(xx_my[:], xx_row[:, ds(off, HALF)])

            # EdgeConv output (my half, channel-major) at static offsets
            if not last:
                xo_my = [
                    lw.tile([t.shape[0], HALF], fp32, tag=f"xo_my{h}",
                            name=f"xo_my{li}_{h}")
                    for h, t in enumerate(yT)
                ]
            else:
                xo_my = x4_my

            # ---- per point-block of my half ----
            for i in range(NBLK):
                xsl = a_my[0:c, ts(i, 128)]  # [c, 128] static-offset slice

                # pd chunks -> packed [128, N] (fused mask|iota from PSUM)
                packed = pB.tile([128, N], fp32, tag="packed")
                for q in range(4):
                    pd_ps = psA.tile([128, 512], fp32, tag="pd_ps")
                    if negxx_full is not None:
                        nc.scalar.copy(pd_ps[:], negxx_full[:, ts(q, 512)])
                        nc.tensor.matmul(
                            pd_ps[:], xsl, bt[:, ts(q, 512)],
                            start=False, stop=True,
                        )
                    else:
                        nc.tensor.matmul(
                            pd_ps[:], a_my[:, ts(i, 128)],
                            bt[:, ts(q, 512)], start=True, stop=True,
                        )
                    nc.vector.scalar_tensor_tensor(
                        packed[:, ts(q, 512)].bitcast(u32),
                        pd_ps[:].bitcast(u32), mask_sb[:],
                        iota_sb[:, ts(q, 512)],
                        op0=OP.bitwise_and, op1=OP.bitwise_or,
                    )

                # chunk-pool top-32
                pool = pS.tile([128, 128], fp32, tag="pool")
                for ch in range(16):
                    nc.vector.max(
                        pool[:, ts(ch, 8)], packed[:, ts(ch, 128)]
                    )
                If_t = pS.tile([128, 32], fp32, tag="If_t")
                for r in range(4):
                    v8 = pS.tile([128, 8], fp32, tag="v8", bufs=8)
                    nc.vector.max(v8[:], pool[:])
                    pos8 = pS.tile([128, 8], u32, tag="pos8", bufs=8)
                    nc.vector.max_index(pos8[:], v8[:], pool[:])
                    # global idx = (pool_pos >> 3) * 128 | (packed & 0x7F)
                    pa = pS.tile([128, 8], u32, tag="pa", bufs=8)
                    nc.vector.tensor_scalar(
                        pa[:], v8[:].bitcast(u32), 127, None,
                        op0=OP.bitwise_and,
                    )
                    pb = pS.tile([128, 8], u32, tag="pb", bufs=8)
                    nc.vector.tensor_scalar(
                        pb[:], pos8[:], 3, 7,
                        op0=OP.logical_shift_right, op1=OP.logical_shift_left,
                    )
                    pc = pS.tile([128, 8], u32, tag="pc", bufs=8)
                    nc.vector.tensor_tensor(pc[:], pb[:], pa[:], op=OP.bitwise_or)
                    nc.vector.tensor_copy(If_t[:, ts(r, 8)], pc[:])
                    if r < 3:
                        nc.vector.match_replace(pool[:], v8[:], pool[:], NEG_BIG)

                # index re-layout for ap_gather
                it_ps = psS.tile([128, 128], fp32, tag="mm128")
                nc.tensor.transpose(it_ps[0:32, :], If_t[:, :], ident_sb[:])
                it_sb = pS.tile([32, 128], fp32, tag="it_sb")
                nc.scalar.copy(it_sb[:], it_ps[0:32, :])
                A_ps = psS.tile([128, 128], fp32, tag="mm128")
                nc.tensor.matmul(A_ps[:], MA_sb[:], it_sb[:], start=True, stop=True)
                B_ps = psS.tile([128, 128], fp32, tag="mm128")
                nc.tensor.matmul(B_ps[:], MB_sb[:], it_sb[:], start=True, stop=True)
                idxs_t = pS.tile([128, 128, 2], i16, tag="idxs_t")
                nc.vector.tensor_copy(idxs_t[:, :, 0:1], A_ps[:, :])
                nc.vector.tensor_copy(idxs_t[:, :, 1:2], B_ps[:, :])

                # gather + reduce + epilogue per output half
                for h in range(nh):
                    ow = yT[h].shape[0]
                    g = pB.tile([128, 128, K], fp32, tag="g")
                    nc.gpsimd.ap_gather(
                        g[0:ow, :, :], yT[h][:, :], idxs_t[0:ow, :, :],
                        channels=ow, num_elems=N, d=1, num_idxs=128 * K,
                    )
                    gmax = pS.tile([128, 128], fp32, tag="gmax", bufs=8)
                    nc.vector.tensor_reduce(
                        gmax[0:ow, :], g[0:ow, :, :], axis=AX.X, op=OP.max
                    )
                    # zT (+bias) and epilogue
                    z_ps = psS.tile([128, 128], fp32, tag="mm128")
                    if li < 3:
                        nc.tensor.matmul(
                            z_ps[0:ow, :], wz_sb[li][h][:],
                            a_my[:, ts(i, 128)], start=True, stop=True,
                        )
                    else:
                        nc.tensor.matmul(
                            z_ps[0:ow, :], wz_sb[li][h][:], xsl,
                            start=True, stop=False,
                        )
                        nc.tensor.matmul(
                            z_ps[0:ow, :], bz4_sb[h][:], ones_row[:],
                            start=False, stop=True,
                        )
                    u_t = pS.tile([128, 128], fp32, tag="u_t", bufs=8)
                    nc.vector.tensor_tensor(
                        u_t[0:ow, :], gmax[0:ow, :], z_ps[0:ow, :], op=OP.add
                    )
                    nc.vector.scalar_tensor_tensor(
                        xo_my[h][ds(0, ow), ts(i, 128)], u_t[0:ow, :], 0.2,
                        u_t[0:ow, :], op0=OP.mult, op1=OP.max,
                    )

            if not last:
                # my half into x_cm[li+1] and the exchange buffer
                for h, t in enumerate(xo_my):
                    ow = t.shape[0]
                    nc.sync.dma_start(
                        x_cm[li + 1][ds(h * 128, ow), ds(off, HALF)], t[:]
                    )
                    nc.sync.dma_start(xchg_in[li][ds(h * 128, ow), :], t[:])
                # ---- pair AllGather; fill the other half of x_cm[li+1] ----
                if n_cores == 1:
                    nc.sync.dma_start(
                        xchg_out[li][0:o, :], xchg_in[li][:, :]
                    )
                    nc.sync.dma_start(
                        xchg_out[li][o:2 * o, :], xchg_in[li][:, :]
                    )
                else:
                    nc.gpsimd.collective_compute(
                        "AllGather",
                        mybir.AluOpType.bypass,
                        replica_groups=groups,
                        ins=[xchg_in[li][:, :]],
                        outs=[xchg_out[li][:, :]],
                    )
                nc.sync.dma_start(
                    x_cm[li + 1][:, ds(other_off, HALF)],
                    xchg_out[li][ds(other_rank * o, o), :],
                )

        for li, (c, o) in enumerate(LAYERS):
            edge_layer(li, c, o)

        # ================= head =================
        psS.release()
        psM.release()
        psA.release()
        pS.release()
        pB.release()
        lw.release()
        w1 = tc.alloc_tile_pool(name="hw1", bufs=1)
        psA2 = tc.alloc_tile_pool(name="hpsA", bufs=2, space="PSUM")
        psC2 = tc.alloc_tile_pool(name="hpsC", bufs=1, space="PSUM")

        # my-half slices of x1..x3 at static offsets (matmul lhsT constraint),
        # cast to bf16: the W5 matmul runs 4x faster and only feeds the
        # terminal head (no KNN compounding).
        x1_f = w1.tile([64, HALF], fp32, tag="x1_f")
        nc.sync.dma_start(x1_f[:], x_cm[1][:, ds(off, HALF)])
        x2_f = w1.tile([64, HALF], fp32, tag="x2_f")
        nc.sync.dma_start(x2_f[:], x_cm[2][:, ds(off, HALF)])
        x3_f = w1.tile([128, HALF], fp32, tag="x3_f")
        nc.sync.dma_start(x3_f[:], x_cm[3][:, ds(off, HALF)])
        x1_my = w1.tile([64, HALF], bf16, tag="x1_my")
        nc.scalar.copy(x1_my[:], x1_f[:])
        x2_my = w1.tile([64, HALF], bf16, tag="x2_my")
        nc.scalar.copy(x2_my[:], x2_f[:])
        x3_my = w1.tile([128, HALF], bf16, tag="x3_my")
        nc.scalar.copy(x3_my[:], x3_f[:])
        x4_b = [w1.tile([128, HALF], bf16, tag=f"x4_b{j}",
                        name=f"x4_b{j}") for j in range(2)]
        for j in range(2):
            nc.scalar.copy(x4_b[j][:], x4_my[j][:])

        w5_sb = []
        for k2, (r0, r1) in enumerate([(0, 64), (64, 128), (128, 256),
                                       (256, 384), (384, 512)]):
            t = w1.tile([r1 - r0, 1024], bf16, tag=f"w5_{k2}")
            nc.sync.dma_start(t[:], w5b[r0:r1, :])
            w5_sb.append(t)

        hmax = w1.tile([128, 1024], fp32, tag="hmax")
        for i in range(8):
            h_ps = psA2.tile([128, 1024], fp32, tag="h_ps")
            lhs = [x1_my[:, ts(i, 128)],
                   x2_my[:, ts(i, 128)],
                   x3_my[:, ts(i, 128)],
                   x4_b[0][:, ts(i, 128)],
                   x4_b[1][:, ts(i, 128)]]
            for q in range(2):
                for ci, l_ap in enumerate(lhs):
                    nc.tensor.matmul(
                        h_ps[:, ts(q, 512)], l_ap,
                        w5_sb[ci][:, ts(q, 512)],
                        start=(ci == 0), stop=(ci == len(lhs) - 1),
                    )
            if i == 0:
                nc.scalar.copy(hmax[:], h_ps[:])
            else:
                nc.vector.tensor_tensor(hmax[:], h_ps[:], hmax[:], op=OP.max)

        # partition-reduce via transposes -> [128, 8] (chan 128*j+p at [p, j])
        hcat = w1.tile([128, 8], fp32, tag="hcat")
        for j in range(8):
            tp = psC2.tile([128, 128], fp32, tag="tp")
            nc.tensor.transpose(tp[:], hmax[:, ts(j, 128)], ident_sb[:])
            nc.vector.tensor_reduce(
                hcat[:, j:j + 1], tp[:], axis=AX.X, op=OP.max
            )
        nc.sync.dma_start(hred_in[:, :], hcat[:])
        if n_cores == 1:
            nc.sync.dma_start(hred_out[:, :], hred_in[:, :])
        else:
            nc.gpsimd.collective_compute(
                "AllReduce", OP.max, replica_groups=groups,
                ins=[hred_in[:, :]], outs=[hred_out[:, :]],
            )
        hfull = w1.tile([128, 8], fp32, tag="hfull")
        nc.sync.dma_start(hfull[:], hred_out[:, :])
        b5_sb = w1.tile([128, 8], fp32, tag="b5_sb")
        nc.sync.dma_start(
            b5_sb[:], b5r.ap().rearrange("o (j p) -> (o p) j", p=128)
        )
        nc.vector.tensor_tensor(hfull[:], hfull[:], b5_sb[:], op=OP.add)
        nc.vector.scalar_tensor_tensor(
            hfull[:], hfull[:], 0.2, hfull[:], op0=OP.mult, op1=OP.max
        )

        # lf / nf columns
        lvec_sb = w1.tile([5, 1], fp32, tag="lvec_sb")
        nc.sync.dma_start(lvec_sb[:], lvec[:, :])
        nvec_sb = w1.tile([7, 1], fp32, tag="nvec_sb")
        nc.sync.dma_start(nvec_sb[:], nvec[:, :])
        w6_sb = w1.tile([5, 64], fp32, tag="w6_sb")
        nc.sync.dma_start(w6_sb[:], w6T[:, :])
        w7_sb = w1.tile([7, 64], fp32, tag="w7_sb")
        nc.sync.dma_start(w7_sb[:], w7T[:, :])
        b6_sb = w1.tile([64, 1], fp32, tag="b6_sb")
        nc.sync.dma_start(b6_sb[:], b6c[:, :])
        b7_sb = w1.tile([64, 1], fp32, tag="b7_sb")
        nc.sync.dma_start(b7_sb[:], b7c[:, :])

        def matvec_col(w_sb, v_sb, b_sb, n_out, tag):
            ps = psC2.tile([n_out, 1], fp32, tag="tpv")
            nc.tensor.matmul(ps[:], w_sb[:], v_sb[:], start=True, stop=True)
            r = w1.tile([n_out, 1], fp32, tag=tag)
            nc.vector.tensor_tensor(r[:], ps[:], b_sb[:], op=OP.add)
            nc.vector.scalar_tensor_tensor(
                r[:], r[:], 0.2, r[:], op0=OP.mult, op1=OP.max
            )
            return r

        lf_sb = matvec_col(w6_sb, lvec_sb, b6_sb, 64, "lf_sb")
        nf_sb = matvec_col(w7_sb, nvec_sb, b7_sb, 64, "nf_sb")

        # u tile [128, 9]: cols 0..7 = h, col 8 = [lf ; nf]
        u_t = w1.tile([128, 9], fp32, tag="u_t")
        nc.vector.tensor_copy(u_t[:, 0:8], hfull[:])
        nc.sync.dma_start(u_t[0:64, 8:9], lf_sb[:])
        nc.sync.dma_start(u_t[64:128, 8:9], nf_sb[:])

        def fc_row(v_cols, n_ch, wT_d, n_out, b_d, relu, tag):
            """out [1, n_out] = v.T @ wT ; v given as [128, n_ch] columns."""
            w_sb = w1.tile([128, n_ch, n_out], fp32, tag=f"{tag}_w")
            nc.sync.dma_start(
                w_sb[:], wT_d.ap().rearrange("(ch p) f -> p ch f", p=128)
            )
            ps = psC2.tile([1, n_out], fp32, tag="fcps")
            for ch in range(n_ch):
                nc.tensor.matmul(
                    ps[:], v_cols[:, ch:ch + 1], w_sb[:, ch, :],
                    start=(ch == 0), stop=(ch == n_ch - 1),
                )
            b_sb = w1.tile([1, n_out], fp32, tag=f"{tag}_b")
            nc.sync.dma_start(b_sb[:], b_d[:, :])
            r = w1.tile([1, n_out], fp32, tag=f"{tag}_r")
            nc.vector.tensor_tensor(r[:], ps[:], b_sb[:], op=OP.add)
            if relu:
                nc.vector.tensor_scalar_max(r[:], r[:], 0.0)
            return r

        def row_to_cols(v_row, n_ch, tag):
            """[1, 128*n_ch] -> [128, n_ch] via PE transposes."""
            cols = w1.tile([128, n_ch], fp32, tag=tag)
            for j in range(n_ch):
                tpv = psC2.tile([128, 1], fp32, tag="tpv2")
                nc.tensor.transpose(
                    tpv[:], v_row[:, ts(j, 128)], ident_sb[0:1, 0:1]
                )
                nc.vector.tensor_copy(cols[:, j:j + 1], tpv[:])
            return cols

        v1 = fc_row(u_t, 9, L1T, 512, b8r, True, "fc1")
        v1c = row_to_cols(v1, 4, "v1c")
        v2 = fc_row(v1c, 4, L2T, 256, b9r, True, "fc2")
        v2c = row_to_cols(v2, 2, "v2c")
        v3 = fc_row(v2c, 2, L3T, 28, bL3r, False, "fc3")
        nc.sync.dma_start(out_t[:, :], v3[:])

        for p in (psC2, psA2, w1, dramp, xcmp, consts):
            p.release()

    nc.compile()
    return nc


_PROGRAM_CACHE = {}


def get_program(n_cores=NCORES):
    key = n_cores
    if key not in _PROGRAM_CACHE:
        _PROGRAM_CACHE[key] = _build_program(n_cores)
    return _PROGRAM_CACHE[key]


def make_in_maps(inputs, n_cores=NCORES):
    """Host-side preprocessing: fold BN into weights, build per-core inputs."""
    f32 = np.float32

    def arr(v):
        return np.ascontiguousarray(np.asarray(v), dtype=f32)

    x = arr(inputs["x"])  # [B, 3, N]
    lmat = arr(inputs["l"])  # [B, 5]
    nmat = arr(inputs["n"])  # [B, 7]

    def fold(g):
        return arr(g) / np.sqrt(f32(1.0) + f32(EPS), dtype=f32)

    common = {}
    for li, (wn, gn, bn) in enumerate(
        [("W1", "g1", "b1"), ("W2", "g2", "b2"), ("W3", "g3", "b3"),
         ("W4", "g4", "b4")]
    ):
        W = arr(inputs[wn])  # [O, 2C]
        s = fold(inputs[gn])
        b = arr(inputs[bn])
        C = W.shape[1] // 2
        Wn = W[:, :C] * s[:, None]
        Wc = W[:, C:] * s[:, None]
        common[f"wy{li}"] = arr(Wn.T)
        wzT = arr((Wc - Wn).T)
        if li < 3:
            common[f"wz{li}"] = arr(np.concatenate(
                [wzT, np.zeros((1, len(b)), f32), b[None, :]], axis=0))
        else:
            common[f"wz{li}"] = wzT
            common["bz3"] = arr(b)[None, :]

    s5 = fold(inputs["g5"])
    import ml_dtypes
    common["w5b"] = np.ascontiguousarray(
        ((arr(inputs["W5"]) * s5[:, None]).T).astype(ml_dtypes.bfloat16))
    common["b5r"] = arr(inputs["b5"])[None, :]
    s6 = fold(inputs["g6"])
    common["w6T"] = arr((arr(inputs["W6"]) * s6[:, None]).T)
    common["b6c"] = arr(inputs["b6"])[:, None]
    s7 = fold(inputs["g7"])
    common["w7T"] = arr((arr(inputs["W7"]) * s7[:, None]).T)
    common["b7c"] = arr(inputs["b7"])[:, None]
    s8 = fold(inputs["g8"])
    common["L1T"] = arr((arr(inputs["L1"]) * s8[:, None]).T)
    common["b8r"] = arr(inputs["b8"])[None, :]
    s9 = fold(inputs["g9"])
    common["L2T"] = arr((arr(inputs["L2"]) * s9[:, None]).T)
    common["b9r"] = arr(s9 * arr(inputs["L2b"]) + arr(inputs["b9"]))[None, :]
    common["L3T"] = arr(arr(inputs["L3"]).T)
    common["bL3r"] = arr(inputs["L3b"])[None, :]
    common["ident"] = np.eye(128, dtype=f32)
    common["iota_pat"] = np.ascontiguousarray(
        np.broadcast_to(np.arange(N, dtype=np.uint32) % 128, (128, N))
    )
    common["mask_col"] = np.full((128, 1), IDX_MASK, dtype=np.uint32)
    MA = np.zeros((32, 128), f32)
    MB = np.zeros((32, 128), f32)
    for r in range(128):
        MA[r % 16, r] = 1.0
        MB[16 + r % 16, r] = 1.0
    common["MA"] = MA
    common["MB"] = MB

    in_maps = []
    for core in range(n_cores):
        b_i = (core // 2) % B
        m = dict(common)
        m["x0"] = arr(x[b_i])
        m["lvec"] = arr(lmat[b_i])[:, None]
        m["nvec"] = arr(nmat[b_i])[:, None]
        in_maps.append(m)
    return in_maps


LAST_RESULTS = None


def kernel(**inputs):
    global LAST_RESULTS
    from concourse.bass_utils import run_bass_kernel_spmd

    nc = get_program(NCORES)
    in_maps = make_in_maps(inputs, NCORES)
    res = run_bass_kernel_spmd(nc, in_maps, core_ids=list(range(NCORES)))
    LAST_RESULTS = res
    rows = [res.results[2 * b]["out"].reshape(28) for b in range(B)]
    return np.stack(rows, axis=0).astype(np.float32)


# revision 26
# speedup vs baseline: 1.1380x; 1.0116x over previous
"""DGCNN (4-layer EdgeConv + head) Bass kernel for 8 Trainium2 NeuronCores.

Problem: nn_DGCNN_net (B=4, N=2048, K=32), eval-mode BN.

Sharding: 2 cores per batch element (B=4 x 2-way split of the N=2048 points).
Cores 2b,2b+1 process batch b; even core owns points 0..1023, odd core
1024..2047. After each of the first three EdgeConv layers the pair exchanges
its half of the new features (pair-wise AllGather); the global max-pool is
combined with a pair-wise AllReduce(max).

v2 design (channel-major, SBUF gather):
  - Everything stays channel-major [chan, points]: y = (s*Wnbr)@x is computed
    as yT [o, N] directly, the neighbor gather runs on GPSIMD via ap_gather
    (SBUF->SBUF free-axis gather, one instruction per 128-point block instead
    of 31 descriptor-generating indirect DMAs), and the EdgeConv output comes
    out channel-major, which is exactly the next layer's input layout - no
    transpose rebuild.
  - k-NN selection: pd[i,j] = 2 x_i.x_j - xx_j (the -xx_i row constant is
    dropped; it does not change the row-wise top-k).  The column index is
    packed into the low 11 mantissa bits of the fp32 pd value
    (packed = (pd & 0xFFFFF800) | j, one fused scalar_tensor_tensor reading
    the PSUM matmul result).  For same-sign floats bit-pattern order ==
    value order, so top-k of packed == top-k of pd quantized to ~2^-12
    relative - and the selected values carry their indices for free.
  - top-32 per row: 16x Max8 over 128-wide chunks -> 128-entry pool, then
    4 rounds of Max8 + 3 MatchReplace on the pool only.  (Exact unless a
    single 128-chunk holds >8 of the row's true top-32: P ~ 7e-4 per row.)
  - The [128,32] u32 index tile is re-laid into ap_gather's wrapped int16
    format [16, 2p+h] with one PE transpose + two selector matmuls + two
    strided fp32->int16 copies.
"""

import numpy as np

EPS = 1e-5
K = 32
N = 2048
B = 4
NCORES = 8
HALF = N // 2
NBLK = HALF // 128  # 8 point-blocks per core

# layer configs: (C_in, O_out)
LAYERS = [(3, 64), (64, 64), (64, 128), (128, 256)]

NEG_BIG = -3.0e38
IDX_MASK = 0xFFFFFF80  # clear low 7 mantissa bits (local idx)


def _build_program(n_cores: int, reduce_on_pool=(False, False, True, True)):
    import concourse.bass as bass
    import concourse.mybir as mybir
    import concourse.bacc as bacc
    import concourse.tile as tile
    from concourse.bass import ds, ts

    fp32 = mybir.dt.float32
    bf16 = mybir.dt.bfloat16
    u32 = mybir.dt.uint32
    i16 = mybir.dt.int16
    AF = mybir.ActivationFunctionType
    OP = mybir.AluOpType
    AX = mybir.AxisListType

    nc = bacc.Bacc(
        "TRN2",
        target_bir_lowering=False,
        debug=False,
        num_devices=n_cores,
    )

    # ---------------- external IO ----------------
    def din(name, shape, dt=fp32):
        return nc.dram_tensor(name, shape, dt, kind="ExternalInput")

    x0 = din("x0", [3, N])
    # per layer: wy [c, o] (NO bias), wz [c, o], bz [o, 1]
    wy_d = [din(f"wy{li}", [c, o]) for li, (c, o) in enumerate(LAYERS)]
    wz_d = [din(f"wz{li}", [c + 2 if li < 3 else c, o])
            for li, (c, o) in enumerate(LAYERS)]
    bz_d = din("bz3", [1, 256])  # L4 bias row (others folded into wz)
    w5b = din("w5b", [512, 1024], mybir.dt.bfloat16)
    b5r = din("b5r", [1, 1024])
    w6T = din("w6T", [5, 64])
    b6c = din("b6c", [64, 1])
    w7T = din("w7T", [7, 64])
    b7c = din("b7c", [64, 1])
    L1T = din("L1T", [1152, 512])
    b8r = din("b8r", [1, 512])
    L2T = din("L2T", [512, 256])
    b9r = din("b9r", [1, 256])
    L3T = din("L3T", [256, 28])
    bL3r = din("bL3r", [1, 28])
    lvec = din("lvec", [5, 1])
    nvec = din("nvec", [7, 1])
    ident = din("ident", [128, 128])
    iota_pat = din("iota_pat", [128, N], u32)   # column index, replicated
    mask_col = din("mask_col", [128, 1], u32)   # 0xFFFFF800
    MA_d = din("MA", [32, 128])                 # selector: row == r%16
    MB_d = din("MB", [32, 128])                 # selector: row == 16 + r%16

    out_t = nc.dram_tensor("out", [1, 28], fp32, kind="ExternalOutput")

    groups = [[2 * i, 2 * i + 1] for i in range(max(1, n_cores // 2))]

    with tile.TileContext(nc) as tc:
        pid = nc.partition_id()
        off = (pid & 1) * HALF          # this core's first point column
        other_off = HALF - off          # the pair core's first point column
        other_rank = 1 - (pid & 1)

        # ---------------- pools ----------------
        consts = tc.alloc_tile_pool(name="consts", bufs=1)
        xcmp = tc.alloc_tile_pool(name="xcmp", bufs=1)
        dramp = tc.alloc_tile_pool(name="dramp", bufs=1, space="DRAM")
        lw = tc.alloc_tile_pool(name="lw", bufs=1)      # layer-wide tiles
        pB = tc.alloc_tile_pool(name="pB", bufs=2)      # big per-block tiles
        pS = tc.alloc_tile_pool(name="pS", bufs=4)      # small per-block tiles
        psA = tc.alloc_tile_pool(name="psA", bufs=2, space="PSUM")  # pd chunks
        psM = tc.alloc_tile_pool(name="psM", bufs=2, space="PSUM")  # [128,512]
        psS = tc.alloc_tile_pool(name="psS", bufs=2, space="PSUM")  # [128,128]

        # ---------------- persistent consts ----------------
        ident_sb = consts.tile([128, 128], fp32, name="ident_sb")
        nc.sync.dma_start(ident_sb[:], ident[:, :])
        iota_sb = consts.tile([128, N], u32, name="iota_sb")
        nc.sync.dma_start(iota_sb[:], iota_pat[:, :])
        mask_sb = consts.tile([128, 1], u32, name="mask_sb")
        nc.sync.dma_start(mask_sb[:], mask_col[:, :])
        MA_sb = consts.tile([32, 128], fp32, name="MA_sb")
        nc.sync.dma_start(MA_sb[:], MA_d[:, :])
        MB_sb = consts.tile([32, 128], fp32, name="MB_sb")
        nc.sync.dma_start(MB_sb[:], MB_d[:, :])
        ones_col = consts.tile([128, 1], fp32, name="ones_col")
        nc.vector.memset(ones_col[:], 1.0)
        ones_row = consts.tile([1, 128], fp32, name="ones_row")
        nc.vector.memset(ones_row[:], 1.0)
        ones_half = consts.tile([1, HALF], fp32, name="ones_half")
        nc.vector.memset(ones_half[:], 1.0)
        neg1_row = consts.tile([1, N], fp32, name="neg1_row")
        nc.vector.memset(neg1_row[:], -1.0)

        # weights: L4 split into two 128-column halves
        wy_sb, wz_sb = [], []
        for li, (c, o) in enumerate(LAYERS):
            zr = c + 2 if li < 3 else c  # wz rows: [(Wc-Wn); 0; b] for L1-3
            if o <= 128:
                t1 = consts.tile([c, o], fp32, name=f"wy_sb{li}")
                nc.sync.dma_start(t1[:], wy_d[li][:, :])
                wy_sb.append([t1])
                t2 = consts.tile([zr, o], fp32, name=f"wz_sb{li}")
                nc.sync.dma_start(t2[:], wz_d[li][:, :])
                wz_sb.append([t2])
            else:
                ys, zs = [], []
                for h in range(o // 128):
                    t1 = consts.tile([c, 128], fp32, name=f"wy_sb{li}_{h}")
                    nc.sync.dma_start(t1[:], wy_d[li][:, ts(h, 128)])
                    ys.append(t1)
                    t2 = consts.tile([zr, 128], fp32, name=f"wz_sb{li}_{h}")
                    nc.sync.dma_start(t2[:], wz_d[li][:, ts(h, 128)])
                    zs.append(t2)
                wy_sb.append(ys)
                wz_sb.append(zs)
        bz4_sb = [consts.tile([1, 128], fp32, name=f"bz4_sb{h}")
                  for h in range(2)]
        for h in range(2):
            nc.sync.dma_start(bz4_sb[h][:], bz_d[:, ts(h, 128)])

        # channel-major layer inputs (x_cm[1..3] double as x1..x3 for head)
        x_cm = [
            xcmp.tile([c, N], fp32, name=f"x_cm{li}")
            for li, (c, o) in enumerate(LAYERS)
        ]
        # my-half-first column order: cols [0:HALF) = my points,
        # [HALF:N) = the pair core's points.  Kills register-offset copies
        # and lets the my-half preamble run before the exchange lands.
        nc.sync.dma_start(x_cm[0][:, 0:HALF], x0[:, ds(off, HALF)])
        nc.sync.dma_start(x_cm[0][:, ds(other_off, HALF)], x0[:, ds(off, HALF)])
        nc.sync.dma_start(x_cm[0][:, HALF:N], x0[:, ds(other_off, HALF)])
        # layer-4 output, my half, channel-major
        x4_my = [xcmp.tile([128, HALF], fp32, name=f"x4_my{j}") for j in range(2)]

        # ---------------- DRAM scratch ----------------
        xchg_in = [
            dramp.tile([o, HALF], fp32, name=f"xchg_in{li}")
            for li, (c, o) in enumerate(LAYERS[:3])
        ]
        xchg_out = [
            dramp.tile([2 * o, HALF], fp32, name=f"xchg_out{li}")
            for li, (c, o) in enumerate(LAYERS[:3])
        ]
        hred_in = dramp.tile([128, 8], fp32, name="hred_in")
        hred_out = dramp.tile([128, 8], fp32, name="hred_out")

        # ================= EdgeConv layers =================
        def edge_layer(li, c, o, xo_prev):
            last = li == 3
            xc = x_cm[li]
            nh = len(wy_sb[li])  # number of 128-wide output halves

            # ---- layer-wide: xx row, -xx bt row, yT, negxx_full ----
            # q-chunks 0,1 (my half) read xo_prev (ready pre-exchange);
            # chunks 2,3 read x_cm (waits on the exchange load).
            def chunk_src(q):
                if xo_prev is None or q >= 2:
                    return xc[:, ts(q, 512)]
                return xo_prev[:, ts(q, 512)]

            xx_row = lw.tile([1, N], fp32, tag="xx_row")
            for q in range(4):
                xsq = lw.tile([c, 512], fp32, tag="xsq", bufs=2)
                nc.scalar.activation(xsq[:], chunk_src(q), AF.Square)
                mm = psM.tile([128, 512], fp32, tag="mm512")
                nc.tensor.matmul(
                    mm[0:1, :], ones_col[0:c, :], xsq[:], start=True, stop=True
                )
                nc.scalar.copy(xx_row[:, ts(q, 512)], mm[0:1, :])

            # bt = [2x; -1; -xx] for L1-3 (c+2 <= 128 rows, exact -d^2 with
            # a_my = [x; xx; 1]); L4: bt = 2x only, the -xx_j arrives via a
            # PSUM prefill and -xx_i via a rank-1 accumulate.
            negxx_row = lw.tile([1, N], fp32, tag="negxx_row")
            nc.scalar.activation(negxx_row[:], xx_row[:], AF.Copy, scale=-1.0)
            if c + 2 <= 128:
                bt = lw.tile([c + 2, N], fp32, tag="bt")
                bt_main_rows = c + 2
            else:
                bt = lw.tile([c, N], fp32, tag="bt")
                bt_main_rows = c
            for q in range(4):
                nc.scalar.activation(
                    bt[0:c, ts(q, 512)], chunk_src(q), AF.Copy, scale=2.0
                )
            if bt_main_rows == c + 2:
                nc.sync.dma_start(bt[c:c + 1, :], neg1_row[:])
                nc.sync.dma_start(bt[c + 1:c + 2, :], negxx_row[:])
                negxx_full = None
            else:
                # L4: prefill tile -xx broadcast to 128 partitions via PE
                negxx_full = lw.tile([128, N], fp32, tag="negxx_full")
                for q in range(4):
                    mm = psM.tile([128, 512], fp32, tag="mm512")
                    nc.tensor.matmul(
                        mm[:], ones_row[:], negxx_row[:, ts(q, 512)],
                        start=True, stop=True,
                    )
                    nc.scalar.copy(negxx_full[:, ts(q, 512)], mm[:])

            # yT [o, N] channel-major (nh tiles of <=128 partitions)
            yT = []
            for h in range(nh):
                ow = wy_sb[li][h].shape[1]
                t = lw.tile([ow, N], fp32, tag=f"yT{h}")
                for q in range(4):
                    mm = psM.tile([128, 512], fp32, tag="mm512")
                    nc.tensor.matmul(
                        mm[0:ow, :], wy_sb[li][h][:], chunk_src(q),
                        start=True, stop=True,
                    )
                    nc.scalar.copy(t[:, ts(q, 512)], mm[0:ow, :])
                yT.append(t)

            # pd lhsT: [x; xx; 1] (c+2 rows) for L1-3; L4 uses [x] plus a
            # separate xx_my row (rank-1 -xx_i accumulate).  Copied to
            # static-offset tiles: matmul operands reject register offsets.
            if bt_main_rows == c + 2:
                a_my = lw.tile([bt_main_rows, HALF], fp32, tag="a_my")
                amy_src = xo_prev if xo_prev is not None else xc[:, 0:HALF]
                nc.sync.dma_start(a_my[0:c, :], amy_src[0:c, 0:HALF]
                                  if xo_prev is not None else amy_src)
                nc.sync.dma_start(a_my[c:c + 1, :], xx_row[:, 0:HALF])
                nc.sync.dma_start(a_my[c + 1:c + 2, :], ones_half[:])
            else:
                # L4: plain x rows; read xo_prev directly as lhsT
                a_my = xo_prev

            # EdgeConv output (my half, channel-major) at static offsets
            if not last:
                xo_my = [
                    lw.tile([t.shape[0], HALF], fp32, tag=f"xo_my{h}",
                            name=f"xo_my{li}_{h}")
                    for h, t in enumerate(yT)
                ]
            else:
                xo_my = x4_my

            # ---- per point-block of my half ----
            for i in range(NBLK):
                xsl = a_my[0:c, ts(i, 128)]  # [c, 128] static-offset slice

                # pd chunks -> packed [128, N] (fused mask|iota from PSUM)
                packed = pB.tile([128, N], fp32, tag="packed")
                for q in range(4):
                    pd_ps = psA.tile([128, 512], fp32, tag="pd_ps")
                    if negxx_full is not None:
                        nc.scalar.copy(pd_ps[:], negxx_full[:, ts(q, 512)])
                        nc.tensor.matmul(
                            pd_ps[:], xsl, bt[:, ts(q, 512)],
                            start=False, stop=True,
                        )
                    else:
                        nc.tensor.matmul(
                            pd_ps[:], a_my[:, ts(i, 128)],
                            bt[:, ts(q, 512)], start=True, stop=True,
                        )
                    nc.vector.scalar_tensor_tensor(
                        packed[:, ts(q, 512)].bitcast(u32),
                        pd_ps[:].bitcast(u32), mask_sb[:],
                        iota_sb[:, ts(q, 512)],
                        op0=OP.bitwise_and, op1=OP.bitwise_or,
                    )

                # chunk-pool top-32
                pool = pS.tile([128, 128], fp32, tag="pool")
                for ch in range(16):
                    nc.vector.max(
                        pool[:, ts(ch, 8)], packed[:, ts(ch, 128)]
                    )
                If_t = pS.tile([128, 32], fp32, tag="If_t")
                for r in range(4):
                    v8 = pS.tile([128, 8], fp32, tag="v8", bufs=8)
                    nc.vector.max(v8[:], pool[:])
                    pos8 = pS.tile([128, 8], u32, tag="pos8", bufs=8)
                    nc.vector.max_index(pos8[:], v8[:], pool[:])
                    # global idx = (pool_pos >> 3) * 128 | (packed & 0x7F)
                    pa = pS.tile([128, 8], u32, tag="pa", bufs=8)
                    nc.vector.tensor_scalar(
                        pa[:], v8[:].bitcast(u32), 127, None,
                        op0=OP.bitwise_and,
                    )
                    pb = pS.tile([128, 8], u32, tag="pb", bufs=8)
                    nc.vector.tensor_scalar(
                        pb[:], pos8[:], 3, 7,
                        op0=OP.logical_shift_right, op1=OP.logical_shift_left,
                    )
                    pc = pS.tile([128, 8], u32, tag="pc", bufs=8)
                    nc.vector.tensor_tensor(pc[:], pb[:], pa[:], op=OP.bitwise_or)
                    nc.vector.tensor_copy(If_t[:, ts(r, 8)], pc[:])
                    if r < 3:
                        nc.vector.match_replace(pool[:], v8[:], pool[:], NEG_BIG)

                # index re-layout for ap_gather
                it_ps = psS.tile([128, 128], fp32, tag="mm128")
                nc.tensor.transpose(it_ps[0:32, :], If_t[:, :], ident_sb[:])
                it_sb = pS.tile([32, 128], fp32, tag="it_sb")
                nc.scalar.copy(it_sb[:], it_ps[0:32, :])
                A_ps = psS.tile([128, 128], fp32, tag="mm128")
                nc.tensor.matmul(A_ps[:], MA_sb[:], it_sb[:], start=True, stop=True)
                B_ps = psS.tile([128, 128], fp32, tag="mm128")
                nc.tensor.matmul(B_ps[:], MB_sb[:], it_sb[:], start=True, stop=True)
                idxs_t = pS.tile([128, 128, 2], i16, tag="idxs_t")
                nc.vector.tensor_copy(idxs_t[:, :, 0:1], A_ps[:, :])
                nc.vector.tensor_copy(idxs_t[:, :, 1:2], B_ps[:, :])

                # gather + reduce + epilogue per output half
                for h in range(nh):
                    ow = yT[h].shape[0]
                    g = pB.tile([128, 128, K], fp32, tag="g")
                    nc.gpsimd.ap_gather(
                        g[0:ow, :, :], yT[h][:, :], idxs_t[0:ow, :, :],
                        channels=ow, num_elems=N, d=1, num_idxs=128 * K,
                    )
                    gmax = pS.tile([128, 128], fp32, tag="gmax", bufs=8)
                    nc.vector.tensor_reduce(
                        gmax[0:ow, :], g[0:ow, :, :], axis=AX.X, op=OP.max
                    )
                    # zT (+bias) and epilogue
                    z_ps = psS.tile([128, 128], fp32, tag="mm128")
                    if li < 3:
                        nc.tensor.matmul(
                            z_ps[0:ow, :], wz_sb[li][h][:],
                            a_my[:, ts(i, 128)], start=True, stop=True,
                        )
                    else:
                        nc.tensor.matmul(
                            z_ps[0:ow, :], wz_sb[li][h][:], xsl,
                            start=True, stop=False,
                        )
                        nc.tensor.matmul(
                            z_ps[0:ow, :], bz4_sb[h][:], ones_row[:],
                            start=False, stop=True,
                        )
                    u_t = pS.tile([128, 128], fp32, tag="u_t", bufs=8)
                    nc.vector.tensor_tensor(
                        u_t[0:ow, :], gmax[0:ow, :], z_ps[0:ow, :], op=OP.add
                    )
                    nc.vector.scalar_tensor_tensor(
                        xo_my[h][ds(0, ow), ts(i, 128)], u_t[0:ow, :], 0.2,
                        u_t[0:ow, :], op0=OP.mult, op1=OP.max,
                    )

            if not last:
                # my half into x_cm[li+1] and the exchange buffer
                for h, t in enumerate(xo_my):
                    ow = t.shape[0]
                    nc.sync.dma_start(
                        x_cm[li + 1][ds(h * 128, ow), 0:HALF], t[:]
                    )
                    nc.sync.dma_start(xchg_in[li][ds(h * 128, ow), :], t[:])
                # ---- pair AllGather; fill the other half of x_cm[li+1] ----
                if n_cores == 1:
                    nc.sync.dma_start(
                        xchg_out[li][0:o, :], xchg_in[li][:, :]
                    )
                    nc.sync.dma_start(
                        xchg_out[li][o:2 * o, :], xchg_in[li][:, :]
                    )
                else:
                    nc.gpsimd.collective_compute(
                        "AllGather",
                        mybir.AluOpType.bypass,
                        replica_groups=groups,
                        ins=[xchg_in[li][:, :]],
                        outs=[xchg_out[li][:, :]],
                    )
                nc.sync.dma_start(
                    x_cm[li + 1][:, HALF:N],
                    xchg_out[li][ds(other_rank * o, o), :],
                )
            return xo_my[0] if not last else None

        xo_prev = None
        for li, (c, o) in enumerate(LAYERS):
            xo_prev = edge_layer(li, c, o, xo_prev)

        # ================= head =================
        psS.release()
        psM.release()
        psA.release()
        pS.release()
        pB.release()
        lw.release()
        w1 = tc.alloc_tile_pool(name="hw1", bufs=1)
        psA2 = tc.alloc_tile_pool(name="hpsA", bufs=2, space="PSUM")
        psC2 = tc.alloc_tile_pool(name="hpsC", bufs=1, space="PSUM")

        # my-half slices of x1..x3 at static offsets (matmul lhsT constraint),
        # cast to bf16: the W5 matmul runs 4x faster and only feeds the
        # terminal head (no KNN compounding).
        x1_my = w1.tile([64, HALF], bf16, tag="x1_my")
        nc.scalar.copy(x1_my[:], x_cm[1][:, 0:HALF])
        x2_my = w1.tile([64, HALF], bf16, tag="x2_my")
        nc.scalar.copy(x2_my[:], x_cm[2][:, 0:HALF])
        x3_my = w1.tile([128, HALF], bf16, tag="x3_my")
        nc.scalar.copy(x3_my[:], x_cm[3][:, 0:HALF])
        x4_b = [w1.tile([128, HALF], bf16, tag=f"x4_b{j}",
                        name=f"x4_b{j}") for j in range(2)]
        for j in range(2):
            nc.scalar.copy(x4_b[j][:], x4_my[j][:])

        w5_sb = []
        for k2, (r0, r1) in enumerate([(0, 64), (64, 128), (128, 256),
                                       (256, 384), (384, 512)]):
            t = w1.tile([r1 - r0, 1024], bf16, tag=f"w5_{k2}")
            nc.sync.dma_start(t[:], w5b[r0:r1, :])
            w5_sb.append(t)

        hmax = w1.tile([128, 1024], fp32, tag="hmax")
        for i in range(8):
            h_ps = psA2.tile([128, 1024], fp32, tag="h_ps")
            lhs = [x1_my[:, ts(i, 128)],
                   x2_my[:, ts(i, 128)],
                   x3_my[:, ts(i, 128)],
                   x4_b[0][:, ts(i, 128)],
                   x4_b[1][:, ts(i, 128)]]
            for q in range(2):
                for ci, l_ap in enumerate(lhs):
                    nc.tensor.matmul(
                        h_ps[:, ts(q, 512)], l_ap,
                        w5_sb[ci][:, ts(q, 512)],
                        start=(ci == 0), stop=(ci == len(lhs) - 1),
                    )
            if i == 0:
                nc.scalar.copy(hmax[:], h_ps[:])
            else:
                nc.vector.tensor_tensor(hmax[:], h_ps[:], hmax[:], op=OP.max)

        # partition-reduce via transposes -> [128, 8] (chan 128*j+p at [p, j])
        hcat = w1.tile([128, 8], fp32, tag="hcat")
        for j in range(8):
            tp = psC2.tile([128, 128], fp32, tag="tp")
            nc.tensor.transpose(tp[:], hmax[:, ts(j, 128)], ident_sb[:])
            nc.vector.tensor_reduce(
                hcat[:, j:j + 1], tp[:], axis=AX.X, op=OP.max
            )
        nc.sync.dma_start(hred_in[:, :], hcat[:])
        if n_cores == 1:
            nc.sync.dma_start(hred_out[:, :], hred_in[:, :])
        else:
            nc.gpsimd.collective_compute(
                "AllReduce", OP.max, replica_groups=groups,
                ins=[hred_in[:, :]], outs=[hred_out[:, :]],
            )
        hfull = w1.tile([128, 8], fp32, tag="hfull")
        nc.sync.dma_start(hfull[:], hred_out[:, :])
        b5_sb = w1.tile([128, 8], fp32, tag="b5_sb")
        nc.sync.dma_start(
            b5_sb[:], b5r.ap().rearrange("o (j p) -> (o p) j", p=128)
        )
        nc.vector.tensor_tensor(hfull[:], hfull[:], b5_sb[:], op=OP.add)
        nc.vector.scalar_tensor_tensor(
            hfull[:], hfull[:], 0.2, hfull[:], op0=OP.mult, op1=OP.max
        )

        # lf / nf columns
        lvec_sb = w1.tile([5, 1], fp32, tag="lvec_sb")
        nc.sync.dma_start(lvec_sb[:], lvec[:, :])
        nvec_sb = w1.tile([7, 1], fp32, tag="nvec_sb")
        nc.sync.dma_start(nvec_sb[:], nvec[:, :])
        w6_sb = w1.tile([5, 64], fp32, tag="w6_sb")
        nc.sync.dma_start(w6_sb[:], w6T[:, :])
        w7_sb = w1.tile([7, 64], fp32, tag="w7_sb")
        nc.sync.dma_start(w7_sb[:], w7T[:, :])
        b6_sb = w1.tile([64, 1], fp32, tag="b6_sb")
        nc.sync.dma_start(b6_sb[:], b6c[:, :])
        b7_sb = w1.tile([64, 1], fp32, tag="b7_sb")
        nc.sync.dma_start(b7_sb[:], b7c[:, :])

        def matvec_col(w_sb, v_sb, b_sb, n_out, tag):
            ps = psC2.tile([n_out, 1], fp32, tag="tpv")
            nc.tensor.matmul(ps[:], w_sb[:], v_sb[:], start=True, stop=True)
            r = w1.tile([n_out, 1], fp32, tag=tag)
            nc.vector.tensor_tensor(r[:], ps[:], b_sb[:], op=OP.add)
            nc.vector.scalar_tensor_tensor(
                r[:], r[:], 0.2, r[:], op0=OP.mult, op1=OP.max
            )
            return r

        lf_sb = matvec_col(w6_sb, lvec_sb, b6_sb, 64, "lf_sb")
        nf_sb = matvec_col(w7_sb, nvec_sb, b7_sb, 64, "nf_sb")

        # u tile [128, 9]: cols 0..7 = h, col 8 = [lf ; nf]
        u_t = w1.tile([128, 9], fp32, tag="u_t")
        nc.vector.tensor_copy(u_t[:, 0:8], hfull[:])
        nc.sync.dma_start(u_t[0:64, 8:9], lf_sb[:])
        nc.sync.dma_start(u_t[64:128, 8:9], nf_sb[:])

        def fc_row(v_cols, n_ch, wT_d, n_out, b_d, relu, tag):
            """out [1, n_out] = v.T @ wT ; v given as [128, n_ch] columns."""
            w_sb = w1.tile([128, n_ch, n_out], fp32, tag=f"{tag}_w")
            nc.sync.dma_start(
                w_sb[:], wT_d.ap().rearrange("(ch p) f -> p ch f", p=128)
            )
            ps = psC2.tile([1, n_out], fp32, tag="fcps")
            for ch in range(n_ch):
                nc.tensor.matmul(
                    ps[:], v_cols[:, ch:ch + 1], w_sb[:, ch, :],
                    start=(ch == 0), stop=(ch == n_ch - 1),
                )
            b_sb = w1.tile([1, n_out], fp32, tag=f"{tag}_b")
            nc.sync.dma_start(b_sb[:], b_d[:, :])
            r = w1.tile([1, n_out], fp32, tag=f"{tag}_r")
            nc.vector.tensor_tensor(r[:], ps[:], b_sb[:], op=OP.add)
            if relu:
                nc.vector.tensor_scalar_max(r[:], r[:], 0.0)
            return r

        def row_to_cols(v_row, n_ch, tag):
            """[1, 128*n_ch] -> [128, n_ch] via PE transposes."""
            cols = w1.tile([128, n_ch], fp32, tag=tag)
            for j in range(n_ch):
                tpv = psC2.tile([128, 1], fp32, tag="tpv2")
                nc.tensor.transpose(
                    tpv[:], v_row[:, ts(j, 128)], ident_sb[0:1, 0:1]
                )
                nc.vector.tensor_copy(cols[:, j:j + 1], tpv[:])
            return cols

        v1 = fc_row(u_t, 9, L1T, 512, b8r, True, "fc1")
        v1c = row_to_cols(v1, 4, "v1c")
        v2 = fc_row(v1c, 4, L2T, 256, b9r, True, "fc2")
        v2c = row_to_cols(v2, 2, "v2c")
        v3 = fc_row(v2c, 2, L3T, 28, bL3r, False, "fc3")
        nc.sync.dma_start(out_t[:, :], v3[:])

        for p in (psC2, psA2, w1, dramp, xcmp, consts):
            p.release()

    nc.compile()
    return nc


_PROGRAM_CACHE = {}


def get_program(n_cores=NCORES):
    key = n_cores
    if key not in _PROGRAM_CACHE:
        _PROGRAM_CACHE[key] = _build_program(n_cores)
    return _PROGRAM_CACHE[key]


def make_in_maps(inputs, n_cores=NCORES):
    """Host-side preprocessing: fold BN into weights, build per-core inputs."""
    f32 = np.float32

    def arr(v):
        return np.ascontiguousarray(np.asarray(v), dtype=f32)

    x = arr(inputs["x"])  # [B, 3, N]
    lmat = arr(inputs["l"])  # [B, 5]
    nmat = arr(inputs["n"])  # [B, 7]

    def fold(g):
        return arr(g) / np.sqrt(f32(1.0) + f32(EPS), dtype=f32)

    common = {}
    for li, (wn, gn, bn) in enumerate(
        [("W1", "g1", "b1"), ("W2", "g2", "b2"), ("W3", "g3", "b3"),
         ("W4", "g4", "b4")]
    ):
        W = arr(inputs[wn])  # [O, 2C]
        s = fold(inputs[gn])
        b = arr(inputs[bn])
        C = W.shape[1] // 2
        Wn = W[:, :C] * s[:, None]
        Wc = W[:, C:] * s[:, None]
        common[f"wy{li}"] = arr(Wn.T)
        wzT = arr((Wc - Wn).T)
        if li < 3:
            common[f"wz{li}"] = arr(np.concatenate(
                [wzT, np.zeros((1, len(b)), f32), b[None, :]], axis=0))
        else:
            common[f"wz{li}"] = wzT
            common["bz3"] = arr(b)[None, :]

    s5 = fold(inputs["g5"])
    import ml_dtypes
    common["w5b"] = np.ascontiguousarray(
        ((arr(inputs["W5"]) * s5[:, None]).T).astype(ml_dtypes.bfloat16))
    common["b5r"] = arr(inputs["b5"])[None, :]
    s6 = fold(inputs["g6"])
    common["w6T"] = arr((arr(inputs["W6"]) * s6[:, None]).T)
    common["b6c"] = arr(inputs["b6"])[:, None]
    s7 = fold(inputs["g7"])
    common["w7T"] = arr((arr(inputs["W7"]) * s7[:, None]).T)
    common["b7c"] = arr(inputs["b7"])[:, None]
    s8 = fold(inputs["g8"])
    common["L1T"] = arr((arr(inputs["L1"]) * s8[:, None]).T)
    common["b8r"] = arr(inputs["b8"])[None, :]
    s9 = fold(inputs["g9"])
    common["L2T"] = arr((arr(inputs["L2"]) * s9[:, None]).T)
    common["b9r"] = arr(s9 * arr(inputs["L2b"]) + arr(inputs["b9"]))[None, :]
    common["L3T"] = arr(arr(inputs["L3"]).T)
    common["bL3r"] = arr(inputs["L3b"])[None, :]
    common["ident"] = np.eye(128, dtype=f32)
    common["iota_pat"] = np.ascontiguousarray(
        np.broadcast_to(np.arange(N, dtype=np.uint32) % 128, (128, N))
    )
    common["mask_col"] = np.full((128, 1), IDX_MASK, dtype=np.uint32)
    MA = np.zeros((32, 128), f32)
    MB = np.zeros((32, 128), f32)
    for r in range(128):
        MA[r % 16, r] = 1.0
        MB[16 + r % 16, r] = 1.0
    common["MA"] = MA
    common["MB"] = MB

    in_maps = []
    for core in range(n_cores):
        b_i = (core // 2) % B
        m = dict(common)
        m["x0"] = arr(x[b_i])
        m["lvec"] = arr(lmat[b_i])[:, None]
        m["nvec"] = arr(nmat[b_i])[:, None]
        in_maps.append(m)
    return in_maps


LAST_RESULTS = None


def kernel(**inputs):
    global LAST_RESULTS
    from concourse.bass_utils import run_bass_kernel_spmd

    nc = get_program(NCORES)
    in_maps = make_in_maps(inputs, NCORES)
    res = run_bass_kernel_spmd(nc, in_maps, core_ids=list(range(NCORES)))
    LAST_RESULTS = res
    rows = [res.results[2 * b]["out"].reshape(28) for b in range(B)]
    return np.stack(rows, axis=0).astype(np.float32)
